# revision 1
# baseline (speedup 1.0000x reference)
"""Baichuan sliding-window GQA attention block on 8 trn2 NeuronCores.

Sharding: data-parallel over batch (2) x tensor-parallel over heads (4).
Core c handles batch b=c//4, head group g=c%4 (q heads 4g..4g+3, kv heads
2g..2g+1). Each core computes qkv projection, RoPE, 2-tap causal conv,
windowed attention and a row-sharded o_proj partial; the host sums the 4
partials per batch.

All on-chip tensors live in a transposed (feature, token) layout so the
tensor engine's contraction (partition) axis lines up without transposes:
  scoresT[k,q] = sum_d kT[d,k] qT[d,q];  outT[d,q] = sum_k v[k,d] probsT[k,q]
V alone is flipped to (token, dim) via PE transposes. Matmuls run as
float32r (full PE rate for moving dim >= 256, fp32 storage).
"""

import numpy as np
import ml_dtypes

B, S, H = 2, 2048, 2048
NH, NKV, HD = 16, 8, 128
WINDOW = 1024
THETA = 100000.0
TP = 4                      # tensor-parallel ways (head groups)
QH = NH // TP               # 4 q heads per core
KVH = NKV // TP             # 2 kv heads per core
NCORES = 8
SCALE = 1.0 / float(np.sqrt(HD))
NEG = -1.0e30

_CACHE = {}


def _build_program():
    import concourse.bacc as bacc
    import concourse.mybir as mybir
    import concourse.tile as tile

    f32 = mybir.dt.float32
    f32r = mybir.dt.float32r
    bf16 = mybir.dt.bfloat16
    Exp = mybir.ActivationFunctionType.Exp
    mult = mybir.AluOpType.mult
    add = mybir.AluOpType.add

    nc = bacc.Bacc("TRN2", target_bir_lowering=False, debug=False,
                   enable_asserts=False, num_devices=NCORES)

    hT_d = nc.dram_tensor("hT", [H, S], f32r, kind="ExternalInput")
    wpk_d = nc.dram_tensor("wpk", [H, 1024], f32r, kind="ExternalInput")
    wo_d = nc.dram_tensor("wo", [QH * HD, H], f32r, kind="ExternalInput")
    cs_d = nc.dram_tensor("cs", [128, S], f32, kind="ExternalInput")
    sn_d = nc.dram_tensor("sn", [128, S], f32, kind="ExternalInput")
    cw_d = nc.dram_tensor("cw", [128, 8], f32, kind="ExternalInput")
    msk_d = nc.dram_tensor("msk", [128, 2048], f32, kind="ExternalInput")
    eye_d = nc.dram_tensor("eye", [128, 128], f32, kind="ExternalInput")
    one_d = nc.dram_tensor("one", [128, 128], f32r, kind="ExternalInput")
    yT_d = nc.dram_tensor("yT", [H, S], f32, kind="ExternalOutput")

    NT = S // 256            # 8 token chunks of 256
    NK = H // 128            # 16 contraction tiles

    with tile.TileContext(nc) as tc:
        with (
            tc.tile_pool(name="const", bufs=1) as cp,
            tc.tile_pool(name="persist", bufs=1) as pp,
        ):
            cs_sb = cp.tile([128, S], f32, tag="cs", name="cs")
            sn_sb = cp.tile([128, S], f32, tag="sn", name="sn")
            cw_sb = cp.tile([128, 8], f32, tag="cw", name="cw")
            eye_sb = cp.tile([128, 128], f32, tag="eye", name="eye")
            one_sb = cp.tile([128, 128], f32r, tag="one", name="one")
            msk_sb = cp.tile([128, 2048], f32, tag="msk", name="msk")
            nc.sync.dma_start(out=cs_sb[:], in_=cs_d[:, :])
            nc.sync.dma_start(out=sn_sb[:], in_=sn_d[:, :])
            nc.sync.dma_start(out=cw_sb[:], in_=cw_d[:, :])
            nc.sync.dma_start(out=eye_sb[:], in_=eye_d[:, :])
            nc.sync.dma_start(out=one_sb[:], in_=one_d[:, :])

            # persistent across phases
            qpair = [pp.tile([128, 2 * S], f32r, tag=f"qp{i}", name=f"qp{i}") for i in range(KVH)]
            kconv = [pp.tile([128, S], f32r, tag=f"kc{i}", name=f"kc{i}") for i in range(KVH)]
            vt = [[pp.tile([128, 128], f32r, tag=f"vt{i}_{j}", name=f"vt{i}_{j}") for j in range(NK)]
                  for i in range(KVH)]

            # ---- phase B: fused qkv projection + rope + conv + v-transpose,
            # one f32r pass over hT in 256-token chunks. k/v staged in 2-chunk
            # rolling buffers; conv and the v transpose run per chunk.
            with (
                tc.tile_pool(name="bw", bufs=1) as bw,
                tc.tile_pool(name="bht", bufs=2) as bht,
                tc.tile_pool(name="broll", bufs=1) as br,
                tc.tile_pool(name="btmp", bufs=2) as bt,
                tc.tile_pool(name="bps", bufs=6, space="PSUM") as psb,
                tc.tile_pool(name="bps2", bufs=2, space="PSUM") as pse2,
                tc.tile_pool(name="bpst", bufs=1, space="PSUM") as pst,
            ):
                wf = [bw.tile([128, 1024], f32r, tag=f"wf{k}", name=f"wf{k}")
                      for k in range(NK)]
                hts0 = []
                for k in range(NK):
                    ht = bht.tile([128, 256], f32r, tag=f"ht{k}", name=f"ht{k}")
                    nc.sync.dma_start(out=ht[:],
                                      in_=hT_d[k * 128:(k + 1) * 128, 0:256])
                    hts0.append(ht)
                for k in range(NK):
                    nc.sync.dma_start(out=wf[k][:],
                                      in_=wpk_d[k * 128:(k + 1) * 128, :])
                kbuf = [br.tile([128, 512], f32, name=f"kbuf{i}") for i in range(KVH)]
                vbuf = [br.tile([128, 512], f32, name=f"vbuf{i}") for i in range(KVH)]
                for t in range(NT):
                    cur, prv = (t % 2) * 256, ((t + 1) % 2) * 256
                    if t == 0:
                        hts = hts0
                    else:
                        hts = []
                        for k in range(NK):
                            ht = bht.tile([128, 256], f32r, tag=f"ht{k}",
                                          name=f"ht{k}")
                            nc.sync.dma_start(
                                out=ht[:],
                                in_=hT_d[k * 128:(k + 1) * 128,
                                         t * 256:(t + 1) * 256])
                            hts.append(ht)
                    csl = cs_sb[:, t * 256:(t + 1) * 256]
                    snl = sn_sb[:, t * 256:(t + 1) * 256]
                    if t == 0:
                        # k-outer for the first chunk: 6 open accumulations so
                        # PE advances with each arriving weight tile instead of
                        # gating on the last one
                        psc0 = [psb.tile([128, 256], f32, tag="qkps",
                                         name=f"qk0_{c}") for c in range(6)]
                        for k in range(NK):
                            for c in range(6):
                                nc.tensor.matmul(
                                    psc0[c][:], wf[k][:, c * 128:(c + 1) * 128],
                                    hts[k][:],
                                    start=(k == 0), stop=(k == NK - 1))
                    for col in range(8):
                        if t == 0 and col < 6:
                            ps = psc0[col]
                        else:
                            ps = psb.tile([128, 256], f32, tag="qkps", name="qkps")
                            for k in range(NK):
                                nc.tensor.matmul(ps[:],
                                                 wf[k][:, col * 128:(col + 1) * 128],
                                                 hts[k][:],
                                                 start=(k == 0), stop=(k == NK - 1))
                        if col < 6:
                            e1 = bt.tile([128, 256], f32, tag="e1", name="e1")
                            e2 = pse2.tile([128, 256], f32, tag="e2", name="e2",
                                           bufs=1)
                            nc.vector.tensor_mul(e1[:], ps[:], csl)
                            nc.vector.tensor_mul(e2[:], ps[:], snl)
                            if col < 4:
                                dest = qpair[col // 2]
                                off = (col % 2) * S + t * 256
                            else:
                                dest = kbuf[col - 4]
                                off = cur
                            nc.vector.tensor_sub(dest[0:64, off:off + 256],
                                                 e1[0:64, :], e2[64:128, :])
                            nc.vector.tensor_add(dest[64:128, off:off + 256],
                                                 e2[0:64, :], e1[64:128, :])
                        else:
                            nc.scalar.copy(vbuf[col - 6][:, cur:cur + 256], ps[:])
                    # per-chunk conv (k -> kconv tile, v -> vcb) + v transpose
                    for i in range(KVH):
                        w0k, w1k = cw_sb[:, 2 * i:2 * i + 1], cw_sb[:, 2 * i + 1:2 * i + 2]
                        w0v, w1v = (cw_sb[:, 4 + 2 * i:5 + 2 * i],
                                    cw_sb[:, 5 + 2 * i:6 + 2 * i])
                        kc = kconv[i]
                        tmp = bt.tile([128, 256], f32, tag="ct", name="ct")
                        nc.vector.tensor_scalar_mul(tmp[:], kbuf[i][:, cur:cur + 256],
                                                    w1k)
                        nc.vector.scalar_tensor_tensor(
                            kc[:, t * 256 + 1:t * 256 + 256],
                            kbuf[i][:, cur:cur + 255], w0k, tmp[:, 1:256], mult, add)
                        if t == 0:
                            nc.vector.tensor_copy(kc[:, 0:1], tmp[:, 0:1])
                        else:
                            nc.vector.scalar_tensor_tensor(
                                kc[:, t * 256:t * 256 + 1],
                                kbuf[i][:, prv + 255:prv + 256], w0k,
                                tmp[:, 0:1], mult, add)
                        vcb = bt.tile([128, 256], f32, tag="vcb", name="vcb")
                        tm2 = bt.tile([128, 256], f32, tag="ct2", name="ct2")
                        nc.vector.tensor_scalar_mul(tm2[:], vbuf[i][:, cur:cur + 256],
                                                    w1v)
                        nc.vector.scalar_tensor_tensor(
                            vcb[:, 1:256], vbuf[i][:, cur:cur + 255], w0v,
                            tm2[:, 1:256], mult, add)
                        if t == 0:
                            nc.vector.tensor_copy(vcb[:, 0:1], tm2[:, 0:1])
                        else:
                            nc.vector.scalar_tensor_tensor(
                                vcb[:, 0:1], vbuf[i][:, prv + 255:prv + 256], w0v,
                                tm2[:, 0:1], mult, add)
                        for h in range(2):
                            tp = pst.tile([128, 128], f32, tag="vtp", name="vtp")
                            nc.tensor.transpose(tp[:], vcb[:, h * 128:(h + 1) * 128],
                                                eye_sb[:])
                            nc.vector.tensor_copy(vt[i][2 * t + h][:], tp[:])

            nc.sync.dma_start(out=msk_sb[:], in_=msk_d[:, :])
            # ---- phase E: banded attention;  phase F: o_proj partial ----
            with (
                tc.tile_pool(name="att", bufs=1) as ap,
                tc.tile_pool(name="atw", bufs=3) as aw,
            ):
                wo_sb = [ap.tile([128, H], f32r, tag=f"wo{d}", name=f"wo{d}")
                         for d in range(QH)]
                for d in range(QH):
                    nc.sync.dma_start(out=wo_sb[d][:],
                                      in_=wo_d[d * 128:(d + 1) * 128, :])
                attn = [ap.tile([128, S], f32r, tag=f"at{h}", name=f"at{h}")
                        for h in range(QH)]

                with (
                    tc.tile_pool(name="eps_sc", bufs=3, space="PSUM") as pss,
                    tc.tile_pool(name="eps_pv", bufs=2, space="PSUM") as psv,
                    tc.tile_pool(name="eps_sm", bufs=1, space="PSUM") as psm,
                    tc.tile_pool(name="fps", bufs=2, space="PSUM") as psf,
                ):
                  for qi in range(NT):
                    for i in range(KVH):
                        qc = qi * 256
                        jstart = max(0, qc // 128 - 8)
                        jend = qc // 128 + 1
                        ps_o = psv.tile([128, 512], f32, tag="pv", name="pv")
                        ps_s = psm.tile([1, 512], f32, tag="sm", name="sm")
                        jorder = list(range(jstart, jend + 1))
                        if qc - jstart * 128 == 1024:
                            # window-edge tile is half-masked; run it second so
                            # a full tile carries the start=True psum clear
                            jorder[0], jorder[1] = jorder[1], jorder[0]
                        jfirst = jorder[0]
                        for j in jorder:
                            ps_sc = pss.tile([128, 512], f32, tag="sc", name="sc")
                            lhs = kconv[i][:, j * 128:(j + 1) * 128]
                            q2 = qpair[i][:].rearrange("p (h s) -> p h s", h=2)
                            nc.tensor.matmul(
                                ps_sc[:], lhs,
                                q2[:, :, qc:qc + 256],
                                start=True, stop=True)
                            mt = {1024: 0, 896: 1, 0: 2, -128: 3}.get(qc - j * 128)
                            pb = aw.tile([128, 512], f32r, tag="pb", name="pb",
                                         bufs=6)
                            if j != jfirst and qc - j * 128 == 1024:
                                # window-edge tile: second q-half of each head is
                                # fully masked -> valid halves only (never the
                                # start matmul, so the psum clear is full)
                                ps3 = ps_sc.rearrange("p (h q) -> p h q", h=2)
                                pb3 = pb.rearrange("p (h q) -> p h q", h=2)
                                po3 = ps_o.rearrange("p (h q) -> p h q", h=2)
                                pss3 = ps_s.rearrange("p (h q) -> p h q", h=2)
                                mk3 = msk_sb[:, 0:512].rearrange(
                                    "p (h q) -> p h q", h=2)
                                tm = aw.tile([128, 512], f32, tag="tm", name="tm")
                                tm3 = tm.rearrange("p (h q) -> p h q", h=2)
                                nc.vector.tensor_add(
                                    tm3[:, :, 0:128], ps3[:, :, 0:128],
                                    mk3[:, :, 0:128])
                                nc.scalar.activation(pb3[:, :, 0:128],
                                                     tm3[:, :, 0:128], Exp,
                                                     bias=0.0, scale=SCALE)
                                nc.tensor.matmul(po3[:, :, 0:128], vt[i][j][:],
                                                 pb3[:, :, 0:128],
                                                 start=False, stop=False)
                                nc.tensor.matmul(pss3[:, :, 0:128],
                                                 one_sb[:, 0:1],
                                                 pb3[:, :, 0:128],
                                                 start=False, stop=False)
                                continue
                            if j == jend:
                                # delta=-128 tile: first q-half of each head is
                                # fully masked -> process only the valid halves
                                # via a 2-segment AP (n=256 keeps f32r rate)
                                ps3 = ps_sc.rearrange("p (h q) -> p h q", h=2)
                                pb3 = pb.rearrange("p (h q) -> p h q", h=2)
                                po3 = ps_o.rearrange("p (h q) -> p h q", h=2)
                                pss3 = ps_s.rearrange("p (h q) -> p h q", h=2)
                                mk3 = msk_sb[:, mt * 512:(mt + 1) * 512].rearrange(
                                    "p (h q) -> p h q", h=2)
                                tm = aw.tile([128, 512], f32, tag="tm", name="tm")
                                tm3 = tm.rearrange("p (h q) -> p h q", h=2)
                                nc.vector.tensor_add(
                                    tm3[:, :, 128:256], ps3[:, :, 128:256],
                                    mk3[:, :, 128:256])
                                nc.scalar.activation(pb3[:, :, 128:256],
                                                     tm3[:, :, 128:256], Exp,
                                                     bias=0.0, scale=SCALE)
                                nc.tensor.matmul(po3[:, :, 128:256], vt[i][j][:],
                                                 pb3[:, :, 128:256],
                                                 start=False, stop=True)
                                nc.tensor.matmul(pss3[:, :, 128:256],
                                                 one_sb[:, 0:1],
                                                 pb3[:, :, 128:256],
                                                 start=False, stop=True)
                                continue
                            if mt is None:
                                nc.scalar.activation(pb[:], ps_sc[:], Exp,
                                                     bias=0.0, scale=SCALE)
                            else:
                                tm = aw.tile([128, 512], f32, tag="tm", name="tm")
                                nc.vector.tensor_add(
                                    tm[:], ps_sc[:],
                                    msk_sb[:, mt * 512:(mt + 1) * 512])
                                nc.scalar.activation(pb[:], tm[:], Exp,
                                                     bias=0.0, scale=SCALE)
                            nc.tensor.matmul(ps_o[:], vt[i][j][:], pb[:],
                                             start=(j == jfirst), stop=(j == jend))
                            nc.tensor.matmul(ps_s[:], one_sb[:, 0:1], pb[:],
                                             start=(j == jfirst), stop=(j == jend))
                        rsum = aw.tile([1, 512], f32, tag="rs", name="rs")
                        nc.vector.reciprocal(rsum[:], ps_s[:])
                        rb = aw.tile([128, 512], f32, tag="rb", name="rb")
                        nc.gpsimd.partition_broadcast(rb[:], rsum[:])
                        nc.vector.tensor_mul(attn[2 * i][:, qc:qc + 256],
                                             ps_o[:, 0:256], rb[:, 0:256])
                        nc.vector.tensor_mul(attn[2 * i + 1][:, qc:qc + 256],
                                             ps_o[:, 256:512], rb[:, 256:512])

                  for t4 in range(4):
                    for oc in range(NK):
                        ps_y = psf.tile([128, 512], f32, tag="y", name="y")
                        for d in range(QH):
                            nc.tensor.matmul(
                                ps_y[:],
                                wo_sb[d][:, oc * 128:(oc + 1) * 128],
                                attn[d][:, t4 * 512:(t4 + 1) * 512],
                                start=(d == 0), stop=(d == QH - 1))
                        yb = aw.tile([128, 512], f32, tag="yb", name="yb",
                                     bufs=4)
                        if (oc + t4) % 2 == 0:
                            nc.vector.tensor_copy(yb[:], ps_y[:])
                        else:
                            nc.scalar.copy(yb[:], ps_y[:])
                        nc.sync.dma_start(
                            out=yT_d[oc * 128:(oc + 1) * 128,
                                     t4 * 512:(t4 + 1) * 512],
                            in_=yb[:])

    nc.finalize()
    return nc


def _host_inputs(hidden, W_pack, W_o, conv_k, conv_v):
    """Per-core input maps."""
    pos = np.arange(S, dtype=np.float64)
    inv_freq = 1.0 / (THETA ** (np.arange(0, HD, 2, dtype=np.float64) / HD))
    freqs = np.outer(pos, inv_freq)                       # (S, 64)
    cos = np.cos(freqs).T.astype(np.float32)              # (64, S)
    sin = np.sin(freqs).T.astype(np.float32)
    cs = np.concatenate([cos, cos], axis=0)               # (128, S)
    sn = np.concatenate([sin, sin], axis=0)

    kk = np.arange(128)[:, None]
    qq = np.arange(256)[None, :]
    def double(m):
        return np.concatenate([m, m], axis=1).astype(np.float32)
    t0 = double(np.where(kk <= qq, 0.0, NEG))             # delta = 0
    tm128 = double(np.where(kk <= qq - 128, 0.0, NEG))    # delta = -128
    w896 = double(np.where(qq - kk < 128, 0.0, NEG))      # delta = 896
    w1024 = double(np.where(qq < kk, 0.0, NEG))           # delta = 1024
    msk = np.concatenate([w1024, w896, t0, tm128], axis=1)  # (128, 2048)

    eye = np.eye(128, dtype=np.float32)
    one = np.ones((128, 128), dtype=np.float32)

    in_maps = []
    for c in range(NCORES):
        b, g = c // TP, c % TP
        hT = np.ascontiguousarray(hidden[b].T).astype(np.float32)
        wq = W_pack[:, g * 512:(g + 1) * 512]
        wk = W_pack[:, NH * HD + 2 * g * 128: NH * HD + (2 * g + 2) * 128]
        wv = W_pack[:, NH * HD + NKV * HD + 2 * g * 128:
                    NH * HD + NKV * HD + (2 * g + 2) * 128]
        wpk = np.ascontiguousarray(
            np.concatenate([wq, wk, wv], axis=1)).astype(np.float32)
        wo = np.ascontiguousarray(
            W_o[g * 512:(g + 1) * 512, :]).astype(np.float32)
        cwv = np.empty(8, np.float32)
        for i in range(KVH):
            cwv[2 * i] = conv_k[2 * g + i, 0]
            cwv[2 * i + 1] = conv_k[2 * g + i, 1]
            cwv[4 + 2 * i] = conv_v[2 * g + i, 0]
            cwv[4 + 2 * i + 1] = conv_v[2 * g + i, 1]
        cw = np.broadcast_to(cwv, (128, 8)).copy()
        in_maps.append({
            "hT": hT, "wpk": wpk, "wo": wo, "cs": cs, "sn": sn,
            "cw": cw, "msk": msk, "eye": eye, "one": one,
        })
    return in_maps


def run_cores(in_maps, trace=False, **kw):
    from concourse.bass_utils import run_bass_kernel_spmd
    if "nc" not in _CACHE:
        _CACHE["nc"] = _build_program()
    return run_bass_kernel_spmd(_CACHE["nc"], in_maps, list(range(NCORES)),
                                trace=trace, **kw)


def kernel(hidden, W_pack, W_o, conv_k, conv_v):
    hidden = np.asarray(hidden, np.float32)
    W_pack = np.asarray(W_pack, np.float32)
    W_o = np.asarray(W_o, np.float32)
    conv_k = np.asarray(conv_k, np.float32)
    conv_v = np.asarray(conv_v, np.float32)
    in_maps = _host_inputs(hidden, W_pack, W_o, conv_k, conv_v)
    res = run_cores(in_maps)
    out = np.zeros((B, S, H), np.float32)
    for c in range(NCORES):
        b = c // TP
        out[b] += res.results[c]["yT"].T
    return out



# revision 12
# speedup vs baseline: 1.0137x; 1.0137x over previous
"""Baichuan sliding-window GQA attention block on 8 trn2 NeuronCores.

Sharding: data-parallel over batch (2) x tensor-parallel over heads (4).
Core c handles batch b=c//4, head group g=c%4 (q heads 4g..4g+3, kv heads
2g..2g+1). Each core computes qkv projection, RoPE, 2-tap causal conv,
windowed attention and a row-sharded o_proj partial; the host sums the 4
partials per batch.

v2: software-pipelined chunk loop. Iteration t emits, interleaved at the
instruction level: qkv matmuls for chunk t, attention tile-steps for
chunk t-1, and o_proj blocks for chunk t-2 — so the PE always has
independent GEMM work between an attention tile's scores matmul (PE) ->
exp (ACT) -> mask/fold (DVE) -> pv matmul (PE) chain. bf16 storage for
all DMA'd/static tensors (PE 1 cyc/row, DMA halved, DVE 2-4x modes).
V is projected transposed (hT stationary) so it lands in [token, dim]
layout and needs no PE transposes; conv w1 is folded into W_k/W_v
host-side (rope is linear) making the conv one scalar_tensor_tensor per
head; masking is multiplicative post-exp; softmax denominators come from
bf16 DVE tile-folds + a single [1,512] ones-matmul per (chunk, kv head).

Layouts (per core, on-chip):
  qpair[i] [128d, 2*S]   roped Q, the 2 q-heads sharing kv head i
  kconv[i] [128d, S]     roped+conv'd K (pre-scaled by w1k via weights)
  vt[i][j] [128tok,128d] conv'd V tile for token block j (w1v in weights)
  scoresT[k,q] = sum_d kT[d,k] qT[d,q]; outT[d,q] = sum_k v[k,d] pT[k,q]
"""

import numpy as np
import ml_dtypes

B, S, H = 2, 2048, 2048
NH, NKV, HD = 16, 8, 128
WINDOW = 1024
THETA = 100000.0
TP = 4                      # tensor-parallel ways (head groups)
QH = NH // TP               # 4 q heads per core
KVH = NKV // TP             # 2 kv heads per core
NCORES = 8
SCALE = 1.0 / float(np.sqrt(HD))

NT = S // 256               # 8 token chunks of 256
NK = H // 128               # 16 contraction tiles

_CACHE = {}


def _build_program():
    import concourse.bacc as bacc
    import concourse.mybir as mybir
    import concourse.tile as tile

    f32 = mybir.dt.float32
    bf16 = mybir.dt.bfloat16
    Exp = mybir.ActivationFunctionType.Exp
    mult = mybir.AluOpType.mult
    add = mybir.AluOpType.add

    nc = bacc.Bacc("TRN2", target_bir_lowering=False, debug=False,
                   enable_asserts=False, num_devices=NCORES)

    hT_d = nc.dram_tensor("hT", [H, S], bf16, kind="ExternalInput")
    wpk_d = nc.dram_tensor("wpk", [H, 1024], bf16, kind="ExternalInput")
    wo_d = nc.dram_tensor("wo", [QH * HD, H], bf16, kind="ExternalInput")
    # cos/sin per 256-chunk, duplicated for the 2 head columns of a pair:
    # [128, NT, 2, 256] flattened; loaded chunk-at-a-time
    cs_d = nc.dram_tensor("cs", [128, NT * 512], f32, kind="ExternalInput")
    sn_d = nc.dram_tensor("sn", [128, NT * 512], f32, kind="ExternalInput")
    cw_d = nc.dram_tensor("cw", [128, 4], f32, kind="ExternalInput")
    # multiplicative bf16 masks: [m1024h 256 | m896 512 | m0 512 | mm128h 256]
    msk_d = nc.dram_tensor("msk", [128, 1536], bf16, kind="ExternalInput")
    one_d = nc.dram_tensor("one", [128, 1], bf16, kind="ExternalInput")
    yT_d = nc.dram_tensor("yT", [H, S], f32, kind="ExternalOutput")

    with tile.TileContext(nc) as tc:
        with (
            tc.tile_pool(name="const", bufs=1) as cp,
            tc.tile_pool(name="wts", bufs=1) as wp,
            tc.tile_pool(name="persist", bufs=1) as pp,
            tc.tile_pool(name="ht", bufs=2) as bht,
            tc.tile_pool(name="roll", bufs=2) as rl,
            tc.tile_pool(name="ebuf", bufs=2) as eb,
            tc.tile_pool(name="pb", bufs=4) as pbp,
            tc.tile_pool(name="accp", bufs=2) as accp,
            tc.tile_pool(name="accf", bufs=2) as accfp,
            tc.tile_pool(name="rbp", bufs=2) as rbp,
            tc.tile_pool(name="ybp", bufs=4) as ybp,
            tc.tile_pool(name="mm", bufs=3, space="PSUM") as mm,
            tc.tile_pool(name="scp", bufs=2, space="PSUM") as scp,
            tc.tile_pool(name="pvp", bufs=2, space="PSUM") as pvp,
            tc.tile_pool(name="opp", bufs=1, space="PSUM") as opp,
        ):
            # --- weight + first-chunk loads, interleaved so PE starts early
            wf = [wp.tile([128, 1024], bf16, tag=f"wf{k}", name=f"wf{k}")
                  for k in range(NK)]
            hts = []
            for k in range(NK):
                nc.sync.dma_start(out=wf[k][:],
                                  in_=wpk_d[k * 128:(k + 1) * 128, :])
                ht = bht.tile([128, 256], bf16, tag=f"ht{k}", name=f"ht{k}")
                nc.sync.dma_start(out=ht[:], in_=hT_d[k * 128:(k + 1) * 128,
                                                      0:256])
                hts.append(ht)
            def load_cssn(t):
                csr = rl.tile([128, 512], f32, tag="csr", name="csr")
                snr = rl.tile([128, 512], f32, tag="snr", name="snr")
                nc.sync.dma_start(out=csr[:],
                                  in_=cs_d[:, t * 512:(t + 1) * 512])
                nc.sync.dma_start(out=snr[:],
                                  in_=sn_d[:, t * 512:(t + 1) * 512])
                return csr, snr

            cssn = load_cssn(0)
            cw_sb = cp.tile([128, 4], f32, tag="cw", name="cw")
            msk_sb = cp.tile([128, 1536], bf16, tag="msk", name="msk")
            one_sb = cp.tile([128, 1], bf16, tag="one", name="one")
            nc.sync.dma_start(out=cw_sb[:], in_=cw_d[:, :])
            nc.sync.dma_start(out=msk_sb[:], in_=msk_d[:, :])
            nc.sync.dma_start(out=one_sb[:], in_=one_d[:, :])
            wo_sb = [wp.tile([128, H], bf16, tag=f"wo{d}", name=f"wo{d}")
                     for d in range(QH)]

            # --- persistent activations (bf16)
            qpair = [pp.tile([128, 2 * S], bf16, tag=f"qp{i}", name=f"qp{i}")
                     for i in range(KVH)]
            kconv = [pp.tile([128, S], bf16, tag=f"kc{i}", name=f"kc{i}")
                     for i in range(KVH)]
            vt = [[pp.tile([128, 128], bf16, tag=f"vt{i}_{j}",
                           name=f"vt{i}_{j}") for j in range(NK)]
                  for i in range(KVH)]
            attn = [pp.tile([128, S], bf16, tag=f"at{h}", name=f"at{h}")
                    for h in range(QH)]

            # masks: offsets into msk_sb
            M1024H, M896, M0, MM128H = 0, 256, 768, 1280

            pending = []      # deferred softmax-denominator closures

            def make_sum_step(acc, ps_o, qc, i):
                def emit():
                    ps_s = scp.tile([128, 512], f32, tag="sc", name="sc")
                    nc.tensor.matmul(ps_s[0:1, :], one_sb[:], acc[:],
                                     start=True, stop=True)
                    rsum = eb.tile([1, 512], f32, tag="rs", name="rs")
                    nc.vector.reciprocal(rsum[:], ps_s[0:1, :])
                    rb = rbp.tile([128, 512], f32, tag="rb", name="rb")
                    nc.gpsimd.partition_broadcast(rb[:], rsum[:])
                    nc.gpsimd.tensor_mul(attn[2 * i][:, qc:qc + 256],
                                         ps_o[:, 0:256], rb[:, 0:256])
                    nc.gpsimd.tensor_mul(attn[2 * i + 1][:, qc:qc + 256],
                                         ps_o[:, 256:512], rb[:, 256:512])
                return emit

            def attention_steps(qi):
                """List of step closures for chunk qi's attention. Each step
                is a small emission unit; the caller interleaves them with
                qkv/o_proj matmuls for pipelining."""
                steps = []
                qc = qi * 256
                for i in range(KVH):
                    st = {"acc": None, "ps_o": None, "po3": None}
                    q2 = qpair[i][:].rearrange("p (h s) -> p h s", h=2)
                    jstart = max(0, qc // 128 - 8)
                    jend = qc // 128 + 1
                    fulls = [j for j in range(jstart, jend)
                             if qc - j * 128 != 1024]
                    halves = ([j for j in range(jstart, jend)
                               if qc - j * 128 == 1024] + [jend])
                    tiles = ([("f", j) for j in fulls] +
                             [("h", j) for j in halves])
                    ntile = len(tiles)

                    def a_step(kind, j, idx, st=st, i=i, qc=qc, q2=q2,
                               ntile=ntile):
                        delta = qc - j * 128
                        ps_sc = scp.tile([128, 512], f32, tag="sc", name="sc")
                        if kind == "f":
                            nc.tensor.matmul(
                                ps_sc[:],
                                kconv[i][:, j * 128:(j + 1) * 128],
                                q2[:, :, qc:qc + 256],
                                start=True, stop=True)
                            pb = pbp.tile([128, 512], bf16, tag="pb",
                                          name="pb")
                            if idx == 0 and delta not in (896, 0):
                                # clean first tile: exp straight into acc
                                acc = accp.tile([128, 512], bf16, tag="acc",
                                                name="acc")
                                nc.scalar.activation(acc[:], ps_sc[:], Exp,
                                                     bias=0.0, scale=SCALE)
                                st["acc"] = acc
                                st["pb"] = acc
                                return
                            nc.scalar.activation(pb[:], ps_sc[:], Exp,
                                                 bias=0.0, scale=SCALE)
                            if delta in (896, 0):
                                moff = M896 if delta == 896 else M0
                                if idx == 0:
                                    acc = accp.tile([128, 512], bf16,
                                                    tag="acc", name="acc")
                                    nc.vector.scalar_tensor_tensor(
                                        acc[:], pb[:], 1.0,
                                        msk_sb[:, moff:moff + 512],
                                        mult, mult)
                                    # pv must use masked probs
                                    st["acc"] = acc
                                    st["pb"] = acc
                                    return
                                nc.vector.scalar_tensor_tensor(
                                    pb[:], pb[:], 1.0,
                                    msk_sb[:, moff:moff + 512], mult, mult)
                            nacc = accp.tile([128, 512], bf16, tag="acc",
                                             name="acc")
                            nc.vector.scalar_tensor_tensor(
                                nacc[:], pb[:], 1.0, st["acc"][:], mult, add)
                            st["acc"] = nacc
                            st["pb"] = pb
                        else:
                            qoff = 0 if delta == 1024 else 128
                            moff = M1024H if delta == 1024 else MM128H
                            ps3 = ps_sc[:].rearrange("p (h q) -> p h q", h=2)
                            nc.tensor.matmul(
                                ps3[:, :, 0:128],
                                kconv[i][:, j * 128:(j + 1) * 128],
                                q2[:, :, qc + qoff:qc + qoff + 128],
                                start=True, stop=True)
                            pb = pbp.tile([128, 512], bf16, tag="pb",
                                          name="pb")
                            pb3 = pb[:].rearrange("p (h q) -> p h q", h=2)
                            nc.scalar.activation(pb3[:, :, 0:128],
                                                 ps3[:, :, 0:128], Exp,
                                                 bias=0.0, scale=SCALE)
                            m3 = msk_sb[:, moff:moff + 256].rearrange(
                                "p (h q) -> p h q", h=2)
                            nc.vector.scalar_tensor_tensor(
                                pb3[:, :, 0:128], pb3[:, :, 0:128], 1.0,
                                m3[:, :, :], mult, mult)
                            last_fold = (idx == ntile - 1)
                            pool = accfp if last_fold else accp
                            tag = "accf" if last_fold else "acc"
                            nacc = pool.tile([128, 512], bf16, tag=tag,
                                             name=tag)
                            acc3 = st["acc"][:].rearrange(
                                "p (h q) -> p h q", h=2)
                            nacc3 = nacc[:].rearrange("p (h q) -> p h q", h=2)
                            nc.vector.scalar_tensor_tensor(
                                nacc3[:, :, qoff:qoff + 128],
                                pb3[:, :, 0:128], 1.0,
                                acc3[:, :, qoff:qoff + 128], mult, add)
                            oq = 128 - qoff
                            nc.vector.tensor_copy(
                                nacc3[:, :, oq:oq + 128],
                                acc3[:, :, oq:oq + 128])
                            st["acc"] = nacc
                            st["pb"] = pb

                    def b_step(kind, j, idx, st=st, i=i, ntile=ntile, qc=qc):
                        first = (idx == 0)
                        last = (idx == ntile - 1)
                        if first:
                            st["ps_o"] = pvp.tile([128, 512], f32, tag="pv",
                                                  name="pv")
                            st["po3"] = st["ps_o"][:].rearrange(
                                "p (h q) -> p h q", h=2)
                        pb = st[f"pb{idx}"]
                        if kind == "f":
                            nc.tensor.matmul(st["ps_o"][:], vt[i][j][:],
                                             pb[:], start=first, stop=last)
                        else:
                            delta = qc - j * 128
                            qoff = 0 if delta == 1024 else 128
                            pb3 = pb[:].rearrange("p (h q) -> p h q", h=2)
                            nc.tensor.matmul(
                                st["po3"][:, :, qoff:qoff + 128],
                                vt[i][j][:], pb3[:, :, 0:128],
                                start=False, stop=last)

                    # assemble: a0,a1,[pending sums],b0,a2,b1,a3,...,bn-2,bn-1
                    def make_a(kind, j, idx, a_step=a_step, st=st):
                        def f():
                            a_step(kind, j, idx)
                            st[f"pb{idx}"] = st["pb"]
                        return f

                    def make_b(kind, j, idx, b_step=b_step):
                        return lambda: b_step(kind, j, idx)

                    seq = []
                    seq.append(make_a(*tiles[0], 0))
                    if ntile > 1:
                        seq.append(make_a(*tiles[1], 1))
                    seq.extend(pending)
                    del pending[:]
                    seq.append(make_b(*tiles[0], 0))
                    for idx in range(2, ntile):
                        seq.append(make_a(*tiles[idx], idx))
                        seq.append(make_b(*tiles[idx - 1], idx - 1))
                    if ntile > 1:
                        seq.append(make_b(*tiles[ntile - 1], ntile - 1))

                    def defer_sum(st=st, qc=qc, i=i):
                        pending.append(
                            make_sum_step(st["acc"], st["ps_o"], qc, i))
                    seq.append(defer_sum)
                    steps.extend(seq)
                return steps

            def oproj_units(qi):
                """8 units; unit = 2 oc blocks x 4 accumulating matmuls over
                a [128,256] token block, sharing one psum bank, then one
                psum->sbuf copy + 2 DMAs out."""
                units = []
                for ocp in range(NK // 2):
                    def unit(ocp=ocp, qi=qi):
                        ps_y = opp.tile([128, 512], f32, tag="op", name="op")
                        for half in range(2):
                            oc = 2 * ocp + half
                            for d in range(QH):
                                nc.tensor.matmul(
                                    ps_y[:, half * 256:(half + 1) * 256],
                                    wo_sb[d][:, oc * 128:(oc + 1) * 128],
                                    attn[d][:, qi * 256:(qi + 1) * 256],
                                    start=(d == 0), stop=(d == QH - 1))
                        yb = ybp.tile([128, 512], f32, tag="yb", name="yb")
                        nc.scalar.copy(yb[:], ps_y[:])
                        for half in range(2):
                            oc = 2 * ocp + half
                            nc.sync.dma_start(
                                out=yT_d[oc * 128:(oc + 1) * 128,
                                         qi * 256:(qi + 1) * 256],
                                in_=yb[:, half * 256:(half + 1) * 256])
                    units.append(unit)
                return units

            krprev = [None]
            vslvprev = [None]

            def rope_and_conv(t, psQ, psK, cssn_t):
                csl = cssn_t[0][:]
                snl = cssn_t[1][:]
                kr = rl.tile([128, 512], bf16, tag="kr", name="kr")
                for cp in range(3):
                    src = psQ[cp] if cp < 2 else psK
                    if cp < 2:
                        dst3 = qpair[cp][:].rearrange(
                            "p (h s) -> p h s", h=2)[:, :,
                                                     t * 256:t * 256 + 256]
                    else:
                        dst3 = kr[:].rearrange("p (h s) -> p h s", h=2)
                    e1 = eb.tile([128, 512], f32, tag="e1", name="e1")
                    e2 = eb.tile([128, 512], f32, tag="e2", name="e2")
                    nc.vector.tensor_mul(e1[:], src[:], csl)
                    nc.vector.tensor_mul(e2[:], src[:], snl)
                    e13 = e1[:].rearrange("p (h s) -> p h s", h=2)
                    e23 = e2[:].rearrange("p (h s) -> p h s", h=2)
                    nc.gpsimd.tensor_sub(dst3[0:64, :, :],
                                         e13[0:64, :, :], e23[64:128, :, :])
                    nc.gpsimd.tensor_add(dst3[64:128, :, :],
                                         e23[0:64, :, :], e13[64:128, :, :])
                # K conv: kconv = kr + (w0k/w1k) * kr_prev (w1k in weights)
                for i in range(KVH):
                    r = cw_sb[:, i:i + 1]
                    o = t * 256
                    nc.vector.scalar_tensor_tensor(
                        kconv[i][:, o + 1:o + 256],
                        kr[:, i * 256:i * 256 + 255], r,
                        kr[:, i * 256 + 1:i * 256 + 256], mult, add)
                    if t == 0:
                        nc.vector.tensor_copy(kconv[i][:, 0:1],
                                              kr[:, i * 256:i * 256 + 1])
                    else:
                        nc.vector.scalar_tensor_tensor(
                            kconv[i][:, o:o + 1],
                            krprev[0][:, i * 256 + 255:i * 256 + 256], r,
                            kr[:, i * 256:i * 256 + 1], mult, add)
                krprev[0] = kr

            def vconv(t, psV):
                vslv = rl.tile([1, 256], f32, tag="vslv", name="vslv")
                for i in range(KVH):
                    r = cw_sb[:, 2 + i:3 + i]
                    for sub in range(2):
                        dst = vt[i][2 * t + sub]
                        src = psV[:, sub * 256 + i * 128:
                                  sub * 256 + (i + 1) * 128]
                        nc.vector.scalar_tensor_tensor(
                            dst[1:128, :], src[0:127, :], r[0:127, :],
                            src[1:128, :], mult, add)
                        if sub == 1:
                            nc.vector.scalar_tensor_tensor(
                                dst[0:1, :],
                                psV[127:128, i * 128:(i + 1) * 128],
                                r[0:1, :], src[0:1, :], mult, add)
                        elif t > 0:
                            nc.vector.scalar_tensor_tensor(
                                dst[0:1, :],
                                vslvprev[0][0:1, i * 128:(i + 1) * 128],
                                r[0:1, :], src[0:1, :], mult, add)
                        else:
                            nc.vector.tensor_copy(dst[0:1, :], src[0:1, :])
                nc.vector.tensor_copy(vslv[:], psV[127:128, 256:512])
                vslvprev[0] = vslv

            # ---------------- main software-pipelined loop ----------------
            for t in range(NT + 1):
                steps = attention_steps(t - 1) if t >= 1 else []
                units = oproj_units(t - 2) if t >= 2 else []
                if t < NT:
                    fill = steps + units
                else:
                    # tail: first steps carry pending softmax sums that
                    # o_proj units consume; then alternate for pipelining
                    fill = steps[:4]
                    rest = steps[4:]
                    for u in range(max(len(units), len(rest))):
                        if u < len(units):
                            fill.append(units[u])
                        if u < len(rest):
                            fill.append(rest[u])
                si = 0

                if t < NT:
                    if t + 1 < NT:
                        nhts = []
                        for k in range(NK):
                            ht = bht.tile([128, 256], bf16, tag=f"ht{k}",
                                          name=f"ht{k}")
                            nc.sync.dma_start(
                                out=ht[:],
                                in_=hT_d[k * 128:(k + 1) * 128,
                                         (t + 1) * 256:(t + 2) * 256])
                            nhts.append(ht)
                        ncssn = load_cssn(t + 1)
                    if t == 0:
                        for d in range(QH):
                            nc.sync.dma_start(out=wo_sb[d][:],
                                              in_=wo_d[d * 128:(d + 1) * 128,
                                                       :])
                    # pass A: Q pairs + K pair, k-outer with 3 open psums
                    psQ = [mm.tile([128, 512], f32, tag="qkv",
                                   name=f"psq{cp}") for cp in range(2)]
                    psK = mm.tile([128, 512], f32, tag="qkv", name="psk")
                    for k in range(NK):
                        for cp in range(2):
                            for h in range(2):
                                c = 2 * cp + h
                                nc.tensor.matmul(
                                    psQ[cp][:, h * 256:(h + 1) * 256],
                                    wf[k][:, c * 128:(c + 1) * 128],
                                    hts[k][:],
                                    start=(k == 0), stop=(k == NK - 1))
                        for h in range(2):
                            nc.tensor.matmul(
                                psK[:, h * 256:(h + 1) * 256],
                                wf[k][:, (4 + h) * 128:(5 + h) * 128],
                                hts[k][:],
                                start=(k == 0), stop=(k == NK - 1))
                        # interleave attention/o_proj emission units
                        want = (k + 1) * len(fill) * 2 // (3 * NK)
                        while si < min(want, len(fill)):
                            fill[si]()
                            si += 1
                    # rope Q/K + K conv first: their emitted reads release
                    # the psQ bufs so psV can take one over
                    rope_and_conv(t, psQ, psK, cssn)
                    # pass B: V transposed (hT stationary) into a freed buf
                    psV = mm.tile([128, 512], f32, tag="qkv", name="psv")
                    for k in range(NK):
                        for sub in range(2):
                            nc.tensor.matmul(
                                psV[:, sub * 256:(sub + 1) * 256],
                                hts[k][:, sub * 128:(sub + 1) * 128],
                                wf[k][:, 768:1024],
                                start=(k == 0), stop=(k == NK - 1))
                    vconv(t, psV)
                    if t + 1 < NT:
                        hts = nhts
                        cssn = ncssn
                while si < len(fill):
                    fill[si]()
                    si += 1
            # last pending softmax denominator (chunk 7, i=1), then its o_proj
            for p in pending:
                p()
            del pending[:]
            for u in oproj_units(NT - 1):
                u()

    nc.finalize()
    return nc


def _host_inputs(hidden, W_pack, W_o, conv_k, conv_v):
    """Per-core input maps."""
    bf16 = ml_dtypes.bfloat16
    pos = np.arange(S, dtype=np.float64)
    inv_freq = 1.0 / (THETA ** (np.arange(0, HD, 2, dtype=np.float64) / HD))
    freqs = np.outer(pos, inv_freq)                       # (S, 64)
    cos = np.cos(freqs).T.astype(np.float32)              # (64, S)
    sin = np.sin(freqs).T.astype(np.float32)
    cos = np.concatenate([cos, cos], axis=0)              # (128, S)
    sin = np.concatenate([sin, sin], axis=0)
    # duplicate per chunk for the two head columns: [128, NT, 2, 256]
    cs = np.broadcast_to(
        cos.reshape(128, NT, 1, 256), (128, NT, 2, 256)).reshape(128, -1)
    sn = np.broadcast_to(
        sin.reshape(128, NT, 1, 256), (128, NT, 2, 256)).reshape(128, -1)
    cs = np.ascontiguousarray(cs).astype(np.float32)
    sn = np.ascontiguousarray(sn).astype(np.float32)

    kk = np.arange(128)[:, None]
    qq = np.arange(128)[None, :]
    qq2 = np.arange(256)[None, :]

    def double(m):
        return np.concatenate([m, m], axis=1)
    m1024h = double(qq < kk)                       # [128, 256]
    m896 = double(qq2 - kk < 128)                  # [128, 512]
    m0 = double(qq2 >= kk)                         # [128, 512]
    mm128h = double(qq >= kk)                      # [128, 256]
    msk = np.concatenate([m1024h, m896, m0, mm128h],
                         axis=1).astype(bf16)      # [128, 1536]

    one = np.ones((128, 1), bf16)

    in_maps = []
    for c in range(NCORES):
        b, g = c // TP, c % TP
        hT = np.ascontiguousarray(hidden[b].T).astype(bf16)
        wq = W_pack[:, g * 512:(g + 1) * 512]
        wk = W_pack[:, NH * HD + 2 * g * 128: NH * HD + (2 * g + 2) * 128]
        wv = W_pack[:, NH * HD + NKV * HD + 2 * g * 128:
                    NH * HD + NKV * HD + (2 * g + 2) * 128]
        # fold conv w1 into Wk/Wv (rope is linear; conv comes after rope)
        wk = wk.copy()
        wv = wv.copy()
        for i in range(KVH):
            wk[:, i * 128:(i + 1) * 128] *= conv_k[2 * g + i, 1]
            wv[:, i * 128:(i + 1) * 128] *= conv_v[2 * g + i, 1]
        wpk = np.ascontiguousarray(
            np.concatenate([wq, wk, wv], axis=1)).astype(bf16)
        wo = np.ascontiguousarray(
            W_o[g * 512:(g + 1) * 512, :]).astype(bf16)
        cwv = np.empty(4, np.float32)
        for i in range(KVH):
            cwv[i] = conv_k[2 * g + i, 0] / conv_k[2 * g + i, 1]
            cwv[2 + i] = conv_v[2 * g + i, 0] / conv_v[2 * g + i, 1]
        cw = np.broadcast_to(cwv, (128, 4)).copy()
        in_maps.append({
            "hT": hT, "wpk": wpk, "wo": wo, "cs": cs, "sn": sn,
            "cw": cw, "msk": msk, "one": one,
        })
    return in_maps


def run_cores(in_maps, trace=False, **kw):
    from concourse.bass_utils import run_bass_kernel_spmd
    if "nc" not in _CACHE:
        _CACHE["nc"] = _build_program()
    return run_bass_kernel_spmd(_CACHE["nc"], in_maps, list(range(NCORES)),
                                trace=trace, **kw)


def kernel(hidden, W_pack, W_o, conv_k, conv_v):
    hidden = np.asarray(hidden, np.float32)
    W_pack = np.asarray(W_pack, np.float32)
    W_o = np.asarray(W_o, np.float32)
    conv_k = np.asarray(conv_k, np.float32)
    conv_v = np.asarray(conv_v, np.float32)
    in_maps = _host_inputs(hidden, W_pack, W_o, conv_k, conv_v)
    res = run_cores(in_maps)
    out = np.zeros((B, S, H), np.float32)
    for c in range(NCORES):
        b = c // TP
        out[b] += res.results[c]["yT"].T
    return out


# revision 15
# speedup vs baseline: 1.0582x; 1.0440x over previous
"""Baichuan sliding-window GQA attention block on 8 trn2 NeuronCores.

Sharding: data-parallel over batch (2) x tensor-parallel over heads (4).
Core c handles batch b=c//4, head group g=c%4 (q heads 4g..4g+3, kv heads
2g..2g+1). Each core computes qkv projection, RoPE, 2-tap causal conv,
windowed attention and a row-sharded o_proj partial; the host sums the 4
partials per batch.

v2: software-pipelined chunk loop. Iteration t emits, interleaved at the
instruction level: qkv matmuls for chunk t, attention tile-steps for
chunk t-1, and o_proj blocks for chunk t-2 — so the PE always has
independent GEMM work between an attention tile's scores matmul (PE) ->
exp (ACT) -> mask/fold (DVE) -> pv matmul (PE) chain. bf16 storage for
all DMA'd/static tensors (PE 1 cyc/row, DMA halved, DVE 2-4x modes).
V is projected transposed (hT stationary) so it lands in [token, dim]
layout and needs no PE transposes; conv w1 is folded into W_k/W_v
host-side (rope is linear) making the conv one scalar_tensor_tensor per
head; masking is multiplicative post-exp; softmax denominators come from
bf16 DVE tile-folds + a single [1,512] ones-matmul per (chunk, kv head).

Layouts (per core, on-chip):
  qpair[i] [128d, 2*S]   roped Q, the 2 q-heads sharing kv head i
  kconv[i] [128d, S]     roped+conv'd K (pre-scaled by w1k via weights)
  vt[i][j] [128tok,128d] conv'd V tile for token block j (w1v in weights)
  scoresT[k,q] = sum_d kT[d,k] qT[d,q]; outT[d,q] = sum_k v[k,d] pT[k,q]
"""

import numpy as np
import ml_dtypes

B, S, H = 2, 2048, 2048
NH, NKV, HD = 16, 8, 128
WINDOW = 1024
THETA = 100000.0
TP = 4                      # tensor-parallel ways (head groups)
QH = NH // TP               # 4 q heads per core
KVH = NKV // TP             # 2 kv heads per core
NCORES = 8
SCALE = 1.0 / float(np.sqrt(HD))

NT = S // 256               # 8 token chunks of 256
NK = H // 128               # 16 contraction tiles

_CACHE = {}


def _build_program():
    import concourse.bacc as bacc
    import concourse.mybir as mybir
    import concourse.tile as tile

    f32 = mybir.dt.float32
    bf16 = mybir.dt.bfloat16
    Exp = mybir.ActivationFunctionType.Exp
    mult = mybir.AluOpType.mult
    add = mybir.AluOpType.add

    nc = bacc.Bacc("TRN2", target_bir_lowering=False, debug=False,
                   enable_asserts=False, num_devices=NCORES)

    hT_d = nc.dram_tensor("hT", [H, S], bf16, kind="ExternalInput")
    wpk_d = nc.dram_tensor("wpk", [H, 1024], bf16, kind="ExternalInput")
    wo_d = nc.dram_tensor("wo", [QH * HD, H], bf16, kind="ExternalInput")
    # cos|sin per 256-chunk, each duplicated for the 2 head columns of a
    # pair: per chunk [cs 512 | sn 512]; loaded chunk-at-a-time in one DMA
    csn_d = nc.dram_tensor("csn", [128, NT * 1024], f32,
                           kind="ExternalInput")
    cw_d = nc.dram_tensor("cw", [128, 4], f32, kind="ExternalInput")
    # multiplicative bf16 masks: [m1024h 256 | m896 512 | m0 512 | mm128h 256]
    msk_d = nc.dram_tensor("msk", [128, 1536], bf16, kind="ExternalInput")
    one_d = nc.dram_tensor("one", [128, 1], bf16, kind="ExternalInput")
    yT_d = nc.dram_tensor("yT", [H, S], f32, kind="ExternalOutput")

    with tile.TileContext(nc) as tc:
        with (
            tc.tile_pool(name="const", bufs=1) as cp,
            tc.tile_pool(name="wts", bufs=1) as wp,
            tc.tile_pool(name="persist", bufs=1) as pp,
            tc.tile_pool(name="ht", bufs=2) as bht,
            tc.tile_pool(name="roll", bufs=2) as rl,
            tc.tile_pool(name="ebuf", bufs=2) as eb,
            tc.tile_pool(name="pb", bufs=4) as pbp,
            tc.tile_pool(name="accp", bufs=2) as accp,
            tc.tile_pool(name="accf", bufs=2) as accfp,
            tc.tile_pool(name="rbp", bufs=2) as rbp,
            tc.tile_pool(name="ybp", bufs=4) as ybp,
            tc.tile_pool(name="mm", bufs=4, space="PSUM") as mm,
            tc.tile_pool(name="scp", bufs=2, space="PSUM") as scp,
            tc.tile_pool(name="pvp", bufs=2, space="PSUM") as pvp,
        ):
            # --- weight + first-chunk loads, interleaved so PE starts
            # early: wf in pairs of k-tiles, ht0 in quarters
            wfc = [wp.tile([128, 2048], bf16, tag=f"wfc{p}", name=f"wfc{p}")
                   for p in range(NK // 2)]

            def wfs(k, lo, hi):
                o = (k % 2) * 1024
                return wfc[k // 2][:, o + lo:o + hi]

            htile = bht.tile([128, NK * 256], bf16, tag="htc", name="htc")
            for p in range(NK // 2):
                nc.sync.dma_start(
                    out=wfc[p][:].rearrange("q (k c) -> q k c", k=2),
                    in_=wpk_d[p * 256:(p + 1) * 256, :].rearrange(
                        "(k q) c -> q k c", k=2))
                if p < 4:
                    nc.sync.dma_start(
                        out=htile[:, p * 1024:(p + 1) * 1024].rearrange(
                            "q (k s) -> q k s", k=4),
                        in_=hT_d[p * 512:(p + 1) * 512, 0:256].rearrange(
                            "(k q) s -> q k s", k=4))

            def load_cssn(t):
                csn = rl.tile([128, 1024], f32, tag="csr", name="csr")
                nc.sync.dma_start(out=csn[:],
                                  in_=csn_d[:, t * 1024:(t + 1) * 1024])
                return csn

            cssn = load_cssn(0)
            cw_sb = cp.tile([128, 4], f32, tag="cw", name="cw")
            msk_sb = cp.tile([128, 1536], bf16, tag="msk", name="msk")
            one_sb = cp.tile([128, 1], bf16, tag="one", name="one")
            nc.sync.dma_start(out=cw_sb[:], in_=cw_d[:, :])
            nc.sync.dma_start(out=msk_sb[:], in_=msk_d[:, :])
            nc.sync.dma_start(out=one_sb[:], in_=one_d[:, :])
            wo_sb = wp.tile([128, QH * H], bf16, tag="wo", name="wo")

            # --- persistent activations (bf16)
            qpair = [pp.tile([128, 2 * S], bf16, tag=f"qp{i}", name=f"qp{i}")
                     for i in range(KVH)]
            kconv = [pp.tile([128, S], bf16, tag=f"kc{i}", name=f"kc{i}")
                     for i in range(KVH)]
            vt = [[pp.tile([128, 128], bf16, tag=f"vt{i}_{j}",
                           name=f"vt{i}_{j}") for j in range(NK)]
                  for i in range(KVH)]
            attn = [pp.tile([128, S], bf16, tag=f"at{h}", name=f"at{h}")
                    for h in range(QH)]

            # masks: offsets into msk_sb
            M1024H, M896, M0, MM128H = 0, 256, 768, 1280

            pending = []      # deferred softmax-denominator closures

            def make_sum_step(acc, ps_o, qc, i):
                def emit():
                    ps_s = scp.tile([128, 512], f32, tag="sc", name="sc")
                    nc.tensor.matmul(ps_s[0:1, :], one_sb[:], acc[:],
                                     start=True, stop=True)
                    rsum = eb.tile([1, 512], f32, tag="rs", name="rs")
                    nc.vector.reciprocal(rsum[:], ps_s[0:1, :])
                    rb = rbp.tile([128, 512], f32, tag="rb", name="rb")
                    nc.gpsimd.partition_broadcast(rb[:], rsum[:])
                    nc.gpsimd.tensor_mul(attn[2 * i][:, qc:qc + 256],
                                         ps_o[:, 0:256], rb[:, 0:256])
                    nc.gpsimd.tensor_mul(attn[2 * i + 1][:, qc:qc + 256],
                                         ps_o[:, 256:512], rb[:, 256:512])
                return emit

            def attention_steps(qi):
                """List of step closures for chunk qi's attention. Each step
                is a small emission unit; the caller interleaves them with
                qkv/o_proj matmuls for pipelining."""
                steps = []
                qc = qi * 256
                for i in range(KVH):
                    st = {"acc": None, "ps_o": None, "po3": None}
                    q2 = qpair[i][:].rearrange("p (h s) -> p h s", h=2)
                    jstart = max(0, qc // 128 - 8)
                    jend = qc // 128 + 1
                    fulls = [j for j in range(jstart, jend)
                             if qc - j * 128 != 1024]
                    halves = ([j for j in range(jstart, jend)
                               if qc - j * 128 == 1024] + [jend])
                    tiles = ([("f", j) for j in fulls] +
                             [("h", j) for j in halves])
                    ntile = len(tiles)

                    def a_step(kind, j, idx, st=st, i=i, qc=qc, q2=q2,
                               ntile=ntile):
                        delta = qc - j * 128
                        ps_sc = scp.tile([128, 512], f32, tag="sc", name="sc")
                        if kind == "f":
                            nc.tensor.matmul(
                                ps_sc[:],
                                kconv[i][:, j * 128:(j + 1) * 128],
                                q2[:, :, qc:qc + 256],
                                start=True, stop=True)
                            pb = pbp.tile([128, 512], bf16, tag="pb",
                                          name="pb")
                            if idx == 0 and delta not in (896, 0):
                                # clean first tile: exp straight into acc
                                acc = accp.tile([128, 512], bf16, tag="acc",
                                                name="acc")
                                nc.scalar.activation(acc[:], ps_sc[:], Exp,
                                                     bias=0.0, scale=SCALE)
                                st["acc"] = acc
                                st["pb"] = acc
                                return
                            nc.scalar.activation(pb[:], ps_sc[:], Exp,
                                                 bias=0.0, scale=SCALE)
                            if delta in (896, 0):
                                moff = M896 if delta == 896 else M0
                                if idx == 0:
                                    acc = accp.tile([128, 512], bf16,
                                                    tag="acc", name="acc")
                                    nc.vector.scalar_tensor_tensor(
                                        acc[:], pb[:], 1.0,
                                        msk_sb[:, moff:moff + 512],
                                        mult, mult)
                                    # pv must use masked probs
                                    st["acc"] = acc
                                    st["pb"] = acc
                                    return
                                nc.vector.scalar_tensor_tensor(
                                    pb[:], pb[:], 1.0,
                                    msk_sb[:, moff:moff + 512], mult, mult)
                            nacc = accp.tile([128, 512], bf16, tag="acc",
                                             name="acc")
                            nc.vector.scalar_tensor_tensor(
                                nacc[:], pb[:], 1.0, st["acc"][:], mult, add)
                            st["acc"] = nacc
                            st["pb"] = pb
                        else:
                            qoff = 0 if delta == 1024 else 128
                            moff = M1024H if delta == 1024 else MM128H
                            ps3 = ps_sc[:].rearrange("p (h q) -> p h q", h=2)
                            nc.tensor.matmul(
                                ps3[:, :, 0:128],
                                kconv[i][:, j * 128:(j + 1) * 128],
                                q2[:, :, qc + qoff:qc + qoff + 128],
                                start=True, stop=True)
                            pb = pbp.tile([128, 512], bf16, tag="pb",
                                          name="pb")
                            pb3 = pb[:].rearrange("p (h q) -> p h q", h=2)
                            nc.scalar.activation(pb3[:, :, 0:128],
                                                 ps3[:, :, 0:128], Exp,
                                                 bias=0.0, scale=SCALE)
                            m3 = msk_sb[:, moff:moff + 256].rearrange(
                                "p (h q) -> p h q", h=2)
                            nc.vector.scalar_tensor_tensor(
                                pb3[:, :, 0:128], pb3[:, :, 0:128], 1.0,
                                m3[:, :, :], mult, mult)
                            last_fold = (idx == ntile - 1)
                            pool = accfp if last_fold else accp
                            tag = "accf" if last_fold else "acc"
                            nacc = pool.tile([128, 512], bf16, tag=tag,
                                             name=tag)
                            acc3 = st["acc"][:].rearrange(
                                "p (h q) -> p h q", h=2)
                            nacc3 = nacc[:].rearrange("p (h q) -> p h q", h=2)
                            nc.vector.scalar_tensor_tensor(
                                nacc3[:, :, qoff:qoff + 128],
                                pb3[:, :, 0:128], 1.0,
                                acc3[:, :, qoff:qoff + 128], mult, add)
                            oq = 128 - qoff
                            nc.vector.tensor_copy(
                                nacc3[:, :, oq:oq + 128],
                                acc3[:, :, oq:oq + 128])
                            st["acc"] = nacc
                            st["pb"] = pb

                    def b_step(kind, j, idx, st=st, i=i, ntile=ntile, qc=qc):
                        first = (idx == 0)
                        last = (idx == ntile - 1)
                        if first:
                            st["ps_o"] = pvp.tile([128, 512], f32, tag="pv",
                                                  name="pv")
                            st["po3"] = st["ps_o"][:].rearrange(
                                "p (h q) -> p h q", h=2)
                        pb = st[f"pb{idx}"]
                        if kind == "f":
                            nc.tensor.matmul(st["ps_o"][:], vt[i][j][:],
                                             pb[:], start=first, stop=last)
                        else:
                            delta = qc - j * 128
                            qoff = 0 if delta == 1024 else 128
                            pb3 = pb[:].rearrange("p (h q) -> p h q", h=2)
                            nc.tensor.matmul(
                                st["po3"][:, :, qoff:qoff + 128],
                                vt[i][j][:], pb3[:, :, 0:128],
                                start=False, stop=last)

                    # assemble: a0,a1,[pending sums],b0,a2,b1,a3,...,bn-2,bn-1
                    def make_a(kind, j, idx, a_step=a_step, st=st):
                        def f():
                            a_step(kind, j, idx)
                            st[f"pb{idx}"] = st["pb"]
                        return f

                    def make_b(kind, j, idx, b_step=b_step):
                        return lambda: b_step(kind, j, idx)

                    seq = []
                    seq.append(make_a(*tiles[0], 0))
                    if ntile > 1:
                        seq.append(make_a(*tiles[1], 1))
                    seq.extend(pending)
                    del pending[:]
                    seq.append(make_b(*tiles[0], 0))
                    for idx in range(2, ntile):
                        seq.append(make_a(*tiles[idx], idx))
                        seq.append(make_b(*tiles[idx - 1], idx - 1))
                    if ntile > 1:
                        seq.append(make_b(*tiles[ntile - 1], ntile - 1))

                    def defer_sum(st=st, qc=qc, i=i):
                        pending.append(
                            make_sum_step(st["acc"], st["ps_o"], qc, i))
                    seq.append(defer_sum)
                    steps.extend(seq)
                return steps

            def oproj_units(qi):
                """8 units; unit = 2 oc blocks x 4 accumulating matmuls over
                a [128,256] token block, sharing one psum bank, then one
                psum->sbuf copy + 2 DMAs out."""
                units = []
                for ocp in range(NK // 2):
                    def unit(ocp=ocp, qi=qi):
                        ps_y = mm.tile([128, 512], f32, tag="qkv", name="op")
                        for half in range(2):
                            oc = 2 * ocp + half
                            for d in range(QH):
                                nc.tensor.matmul(
                                    ps_y[:, half * 256:(half + 1) * 256],
                                    wo_sb[:, d * H + oc * 128:
                                          d * H + (oc + 1) * 128],
                                    attn[d][:, qi * 256:(qi + 1) * 256],
                                    start=(d == 0), stop=(d == QH - 1))
                        yb = ybp.tile([128, 512], f32, tag="yb", name="yb")
                        nc.scalar.copy(yb[:], ps_y[:])
                        nc.sync.dma_start(
                            out=yT_d[ocp * 256:(ocp + 1) * 256,
                                     qi * 256:(qi + 1) * 256].rearrange(
                                "(o p) s -> p o s", o=2),
                            in_=yb[:].rearrange("p (o s) -> p o s", o=2))
                    units.append(unit)
                return units

            krprev = [None]
            vslvprev = [None]

            def rope_and_conv(t, psQ, psK, cssn_t):
                csl = cssn_t[:, 0:512]
                snl = cssn_t[:, 512:1024]
                kr = rl.tile([128, 512], bf16, tag="kr", name="kr")
                for cp in range(3):
                    src = psQ[cp] if cp < 2 else psK
                    if cp < 2:
                        dst3 = qpair[cp][:].rearrange(
                            "p (h s) -> p h s", h=2)[:, :,
                                                     t * 256:t * 256 + 256]
                    else:
                        dst3 = kr[:].rearrange("p (h s) -> p h s", h=2)
                    e1 = eb.tile([128, 512], f32, tag="e1", name="e1")
                    e2 = eb.tile([128, 512], f32, tag="e2", name="e2")
                    nc.vector.tensor_mul(e1[:], src[:], csl)
                    nc.vector.tensor_mul(e2[:], src[:], snl)
                    e13 = e1[:].rearrange("p (h s) -> p h s", h=2)
                    e23 = e2[:].rearrange("p (h s) -> p h s", h=2)
                    nc.gpsimd.tensor_sub(dst3[0:64, :, :],
                                         e13[0:64, :, :], e23[64:128, :, :])
                    nc.gpsimd.tensor_add(dst3[64:128, :, :],
                                         e23[0:64, :, :], e13[64:128, :, :])
                # K conv: kconv = kr + (w0k/w1k) * kr_prev (w1k in weights)
                for i in range(KVH):
                    r = cw_sb[:, i:i + 1]
                    o = t * 256
                    nc.vector.scalar_tensor_tensor(
                        kconv[i][:, o + 1:o + 256],
                        kr[:, i * 256:i * 256 + 255], r,
                        kr[:, i * 256 + 1:i * 256 + 256], mult, add)
                    if t == 0:
                        nc.vector.tensor_copy(kconv[i][:, 0:1],
                                              kr[:, i * 256:i * 256 + 1])
                    else:
                        nc.vector.scalar_tensor_tensor(
                            kconv[i][:, o:o + 1],
                            krprev[0][:, i * 256 + 255:i * 256 + 256], r,
                            kr[:, i * 256:i * 256 + 1], mult, add)
                krprev[0] = kr

            def vconv(t, psV):
                vslv = rl.tile([1, 256], f32, tag="vslv", name="vslv")
                for i in range(KVH):
                    r = cw_sb[:, 2 + i:3 + i]
                    for sub in range(2):
                        dst = vt[i][2 * t + sub]
                        src = psV[:, sub * 256 + i * 128:
                                  sub * 256 + (i + 1) * 128]
                        nc.vector.scalar_tensor_tensor(
                            dst[1:128, :], src[0:127, :], r[0:127, :],
                            src[1:128, :], mult, add)
                        if sub == 1:
                            nc.vector.scalar_tensor_tensor(
                                dst[0:1, :],
                                psV[127:128, i * 128:(i + 1) * 128],
                                r[0:1, :], src[0:1, :], mult, add)
                        elif t > 0:
                            nc.vector.scalar_tensor_tensor(
                                dst[0:1, :],
                                vslvprev[0][0:1, i * 128:(i + 1) * 128],
                                r[0:1, :], src[0:1, :], mult, add)
                        else:
                            nc.vector.tensor_copy(dst[0:1, :], src[0:1, :])
                nc.vector.tensor_copy(vslv[:], psV[127:128, 256:512])
                vslvprev[0] = vslv

            # ---------------- main software-pipelined loop ----------------
            for t in range(NT + 1):
                steps = attention_steps(t - 1) if t >= 1 else []
                units = oproj_units(t - 2) if t >= 2 else []
                if t < NT:
                    fill = steps + units
                else:
                    # tail: first steps carry pending softmax sums that
                    # o_proj units consume; then alternate for pipelining
                    fill = steps[:4]
                    rest = steps[4:]
                    for u in range(max(len(units), len(rest))):
                        if u < len(units):
                            fill.append(units[u])
                        if u < len(rest):
                            fill.append(rest[u])
                si = 0

                if t < NT:
                    if t + 1 < NT:
                        nhtile = bht.tile([128, NK * 256], bf16, tag="htc",
                                          name="htc")
                        nc.sync.dma_start(
                            out=nhtile[:].rearrange("p (k s) -> p k s", k=NK),
                            in_=hT_d[:, (t + 1) * 256:(t + 2) * 256].rearrange(
                                "(k p) s -> p k s", k=NK))
                        ncssn = load_cssn(t + 1)
                    if t == 0:
                        nc.sync.dma_start(
                            out=wo_sb[:].rearrange("p (d c) -> p d c", d=QH),
                            in_=wo_d[:, :].rearrange("(d p) c -> p d c",
                                                     d=QH))
                    # pass A: Q pairs + K pair, k-outer with 3 open psums
                    psQ = [mm.tile([128, 512], f32, tag="qkv",
                                   name=f"psq{cp}") for cp in range(2)]
                    psK = mm.tile([128, 512], f32, tag="qkv", name="psk")
                    for k in range(NK):
                        hk = htile[:, k * 256:(k + 1) * 256]
                        for cp in range(2):
                            for h in range(2):
                                c = 2 * cp + h
                                nc.tensor.matmul(
                                    psQ[cp][:, h * 256:(h + 1) * 256],
                                    wfs(k, c * 128, (c + 1) * 128), hk,
                                    start=(k == 0), stop=(k == NK - 1))
                        for h in range(2):
                            nc.tensor.matmul(
                                psK[:, h * 256:(h + 1) * 256],
                                wfs(k, (4 + h) * 128, (5 + h) * 128), hk,
                                start=(k == 0), stop=(k == NK - 1))
                        # interleave attention/o_proj emission units
                        want = (k + 1) * len(fill) * 2 // (3 * NK)
                        while si < min(want, len(fill)):
                            fill[si]()
                            si += 1
                    # rope Q/K + K conv first: their emitted reads release
                    # the psQ bufs so psV can take one over
                    rope_and_conv(t, psQ, psK, cssn)
                    # pass B: V transposed (hT stationary) into a freed buf
                    psV = mm.tile([128, 512], f32, tag="qkv", name="psv")
                    for k in range(NK):
                        for sub in range(2):
                            nc.tensor.matmul(
                                psV[:, sub * 256:(sub + 1) * 256],
                                htile[:, k * 256 + sub * 128:
                                      k * 256 + (sub + 1) * 128],
                                wfs(k, 768, 1024),
                                start=(k == 0), stop=(k == NK - 1))
                    vconv(t, psV)
                    if t + 1 < NT:
                        htile = nhtile
                        cssn = ncssn
                while si < len(fill):
                    fill[si]()
                    si += 1
            # last pending softmax denominator (chunk 7, i=1), then its o_proj
            for p in pending:
                p()
            del pending[:]
            for u in oproj_units(NT - 1):
                u()

    nc.finalize()
    return nc


def _host_inputs(hidden, W_pack, W_o, conv_k, conv_v):
    """Per-core input maps."""
    bf16 = ml_dtypes.bfloat16
    pos = np.arange(S, dtype=np.float64)
    inv_freq = 1.0 / (THETA ** (np.arange(0, HD, 2, dtype=np.float64) / HD))
    freqs = np.outer(pos, inv_freq)                       # (S, 64)
    cos = np.cos(freqs).T.astype(np.float32)              # (64, S)
    sin = np.sin(freqs).T.astype(np.float32)
    cos = np.concatenate([cos, cos], axis=0)              # (128, S)
    sin = np.concatenate([sin, sin], axis=0)
    # per chunk: [cos dup x2 heads (512) | sin dup x2 heads (512)]
    cs = np.broadcast_to(
        cos.reshape(128, NT, 1, 256), (128, NT, 2, 256)).reshape(128, NT, 512)
    sn = np.broadcast_to(
        sin.reshape(128, NT, 1, 256), (128, NT, 2, 256)).reshape(128, NT, 512)
    csn = np.concatenate([cs, sn], axis=2).reshape(128, -1)
    csn = np.ascontiguousarray(csn).astype(np.float32)

    kk = np.arange(128)[:, None]
    qq = np.arange(128)[None, :]
    qq2 = np.arange(256)[None, :]

    def double(m):
        return np.concatenate([m, m], axis=1)
    m1024h = double(qq < kk)                       # [128, 256]
    m896 = double(qq2 - kk < 128)                  # [128, 512]
    m0 = double(qq2 >= kk)                         # [128, 512]
    mm128h = double(qq >= kk)                      # [128, 256]
    msk = np.concatenate([m1024h, m896, m0, mm128h],
                         axis=1).astype(bf16)      # [128, 1536]

    one = np.ones((128, 1), bf16)

    in_maps = []
    for c in range(NCORES):
        b, g = c // TP, c % TP
        hT = np.ascontiguousarray(hidden[b].T).astype(bf16)
        wq = W_pack[:, g * 512:(g + 1) * 512]
        wk = W_pack[:, NH * HD + 2 * g * 128: NH * HD + (2 * g + 2) * 128]
        wv = W_pack[:, NH * HD + NKV * HD + 2 * g * 128:
                    NH * HD + NKV * HD + (2 * g + 2) * 128]
        # fold conv w1 into Wk/Wv (rope is linear; conv comes after rope)
        wk = wk.copy()
        wv = wv.copy()
        for i in range(KVH):
            wk[:, i * 128:(i + 1) * 128] *= conv_k[2 * g + i, 1]
            wv[:, i * 128:(i + 1) * 128] *= conv_v[2 * g + i, 1]
        wpk = np.ascontiguousarray(
            np.concatenate([wq, wk, wv], axis=1)).astype(bf16)
        wo = np.ascontiguousarray(
            W_o[g * 512:(g + 1) * 512, :]).astype(bf16)
        cwv = np.empty(4, np.float32)
        for i in range(KVH):
            cwv[i] = conv_k[2 * g + i, 0] / conv_k[2 * g + i, 1]
            cwv[2 + i] = conv_v[2 * g + i, 0] / conv_v[2 * g + i, 1]
        cw = np.broadcast_to(cwv, (128, 4)).copy()
        in_maps.append({
            "hT": hT, "wpk": wpk, "wo": wo, "csn": csn,
            "cw": cw, "msk": msk, "one": one,
        })
    return in_maps


def run_cores(in_maps, trace=False, **kw):
    from concourse.bass_utils import run_bass_kernel_spmd
    if "nc" not in _CACHE:
        _CACHE["nc"] = _build_program()
    return run_bass_kernel_spmd(_CACHE["nc"], in_maps, list(range(NCORES)),
                                trace=trace, **kw)


def kernel(hidden, W_pack, W_o, conv_k, conv_v):
    hidden = np.asarray(hidden, np.float32)
    W_pack = np.asarray(W_pack, np.float32)
    W_o = np.asarray(W_o, np.float32)
    conv_k = np.asarray(conv_k, np.float32)
    conv_v = np.asarray(conv_v, np.float32)
    in_maps = _host_inputs(hidden, W_pack, W_o, conv_k, conv_v)
    res = run_cores(in_maps)
    out = np.zeros((B, S, H), np.float32)
    for c in range(NCORES):
        b = c // TP
        out[b] += res.results[c]["yT"].T
    return out


# revision 17
# speedup vs baseline: 1.0812x; 1.0217x over previous
"""Baichuan sliding-window GQA attention block on 8 trn2 NeuronCores.

Sharding: data-parallel over batch (2) x tensor-parallel over heads (4).
Core c handles batch b=c//4, head group g=c%4 (q heads 4g..4g+3, kv heads
2g..2g+1). Each core computes qkv projection, RoPE, 2-tap causal conv,
windowed attention and a row-sharded o_proj partial; the host sums the 4
partials per batch.

v2: software-pipelined chunk loop. Iteration t emits, interleaved at the
instruction level: qkv matmuls for chunk t, attention tile-steps for
chunk t-1, and o_proj blocks for chunk t-2 — so the PE always has
independent GEMM work between an attention tile's scores matmul (PE) ->
exp (ACT) -> mask/fold (DVE) -> pv matmul (PE) chain. bf16 storage for
all DMA'd/static tensors (PE 1 cyc/row, DMA halved, DVE 2-4x modes).
V is projected transposed (hT stationary) so it lands in [token, dim]
layout and needs no PE transposes; conv w1 is folded into W_k/W_v
host-side (rope is linear) making the conv one scalar_tensor_tensor per
head; masking is multiplicative post-exp; softmax denominators come from
bf16 DVE tile-folds + a single [1,512] ones-matmul per (chunk, kv head).

Layouts (per core, on-chip):
  qpair[i] [128d, 2*S]   roped Q, the 2 q-heads sharing kv head i
  kconv[i] [128d, S]     roped+conv'd K (pre-scaled by w1k via weights)
  vt[i][j] [128tok,128d] conv'd V tile for token block j (w1v in weights)
  scoresT[k,q] = sum_d kT[d,k] qT[d,q]; outT[d,q] = sum_k v[k,d] pT[k,q]
"""

import numpy as np
import ml_dtypes

B, S, H = 2, 2048, 2048
NH, NKV, HD = 16, 8, 128
WINDOW = 1024
THETA = 100000.0
TP = 4                      # tensor-parallel ways (head groups)
QH = NH // TP               # 4 q heads per core
KVH = NKV // TP             # 2 kv heads per core
NCORES = 8
SCALE = 1.0 / float(np.sqrt(HD))

NT = S // 256               # 8 token chunks of 256
NK = H // 128               # 16 contraction tiles

_CACHE = {}


def _build_program():
    import concourse.bacc as bacc
    import concourse.mybir as mybir
    import concourse.tile as tile

    f32 = mybir.dt.float32
    bf16 = mybir.dt.bfloat16
    Exp = mybir.ActivationFunctionType.Exp
    mult = mybir.AluOpType.mult
    add = mybir.AluOpType.add

    nc = bacc.Bacc("TRN2", target_bir_lowering=False, debug=False,
                   enable_asserts=False, num_devices=NCORES)

    hT_d = nc.dram_tensor("hT", [H, S], bf16, kind="ExternalInput")
    wpk_d = nc.dram_tensor("wpk", [H, 1024], bf16, kind="ExternalInput")
    wo_d = nc.dram_tensor("wo", [QH * HD, H], bf16, kind="ExternalInput")
    # cos|sin per 256-chunk, each duplicated for the 2 head columns of a
    # pair: per chunk [cs 512 | sn 512]; loaded chunk-at-a-time in one DMA
    csn_d = nc.dram_tensor("csn", [128, NT * 1024], f32,
                           kind="ExternalInput")
    cw_d = nc.dram_tensor("cw", [128, 4], f32, kind="ExternalInput")
    # multiplicative bf16 masks: [m1024h 256 | m896 512 | m0 512 | mm128h 256]
    msk_d = nc.dram_tensor("msk", [128, 1536], bf16, kind="ExternalInput")
    one_d = nc.dram_tensor("one", [128, 1], bf16, kind="ExternalInput")
    yT_d = nc.dram_tensor("yT", [H, S], f32, kind="ExternalOutput")

    with tile.TileContext(nc) as tc:
        with (
            tc.tile_pool(name="const", bufs=1) as cp,
            tc.tile_pool(name="wts", bufs=1) as wp,
            tc.tile_pool(name="persist", bufs=1) as pp,
            tc.tile_pool(name="ht", bufs=2) as bht,
            tc.tile_pool(name="roll", bufs=2) as rl,
            tc.tile_pool(name="ebuf", bufs=2) as eb,
            tc.tile_pool(name="pb", bufs=4) as pbp,
            tc.tile_pool(name="accp", bufs=2) as accp,
            tc.tile_pool(name="accf", bufs=2) as accfp,
            tc.tile_pool(name="rbp", bufs=2) as rbp,
            tc.tile_pool(name="ybp", bufs=4) as ybp,
            tc.tile_pool(name="mm", bufs=4, space="PSUM") as mm,
            tc.tile_pool(name="scp", bufs=2, space="PSUM") as scp,
            tc.tile_pool(name="pvp", bufs=2, space="PSUM") as pvp,
        ):
            # --- weight + first-chunk loads, interleaved so PE starts
            # early: wf in pairs of k-tiles, ht0 in quarters
            wfc = [wp.tile([128, 2048], bf16, tag=f"wfc{p}", name=f"wfc{p}")
                   for p in range(NK // 2)]

            def wfs(k, lo, hi):
                o = (k % 2) * 1024
                return wfc[k // 2][:, o + lo:o + hi]

            htile = bht.tile([128, NK * 256], bf16, tag="htc", name="htc")
            for p in range(NK // 2):
                nc.sync.dma_start(
                    out=wfc[p][:].rearrange("q (k c) -> q k c", k=2),
                    in_=wpk_d[p * 256:(p + 1) * 256, :].rearrange(
                        "(k q) c -> q k c", k=2))
                if p < 4:
                    nc.sync.dma_start(
                        out=htile[:, p * 1024:(p + 1) * 1024].rearrange(
                            "q (k s) -> q k s", k=4),
                        in_=hT_d[p * 512:(p + 1) * 512, 0:256].rearrange(
                            "(k q) s -> q k s", k=4))

            def load_cssn(t):
                csn = rl.tile([128, 1024], f32, tag="csr", name="csr")
                nc.sync.dma_start(out=csn[:],
                                  in_=csn_d[:, t * 1024:(t + 1) * 1024])
                return csn

            cssn = load_cssn(0)
            cw_sb = cp.tile([128, 4], f32, tag="cw", name="cw")
            msk_sb = cp.tile([128, 1536], bf16, tag="msk", name="msk")
            one_sb = cp.tile([128, 1], bf16, tag="one", name="one")
            nc.sync.dma_start(out=cw_sb[:], in_=cw_d[:, :])
            nc.sync.dma_start(out=msk_sb[:], in_=msk_d[:, :])
            nc.sync.dma_start(out=one_sb[:], in_=one_d[:, :])
            wo_sb = wp.tile([128, QH * H], bf16, tag="wo", name="wo")

            # --- persistent activations (bf16)
            qpair = [pp.tile([128, 2 * S], bf16, tag=f"qp{i}", name=f"qp{i}")
                     for i in range(KVH)]
            kct = [[pp.tile([128, 128], bf16, tag=f"kc{i}_{j}",
                            name=f"kc{i}_{j}") for j in range(NK)]
                   for i in range(KVH)]
            vt = [[pp.tile([128, 128], bf16, tag=f"vt{i}_{j}",
                           name=f"vt{i}_{j}") for j in range(NK)]
                  for i in range(KVH)]
            attn = [pp.tile([128, S], bf16, tag=f"at{h}", name=f"at{h}")
                    for h in range(QH)]

            # masks: offsets into msk_sb
            M1024H, M896, M0, MM128H = 0, 256, 768, 1280

            pending = []      # deferred softmax-denominator closures

            def make_sum_step(acc, ps_o, qc, i):
                def emit():
                    ps_s = scp.tile([128, 512], f32, tag="sc", name="sc")
                    nc.tensor.matmul(ps_s[0:1, :], one_sb[:], acc[:],
                                     start=True, stop=True)
                    rsum = eb.tile([1, 512], f32, tag="rs", name="rs")
                    nc.vector.reciprocal(rsum[:], ps_s[0:1, :])
                    rb = rbp.tile([128, 512], f32, tag="rb", name="rb")
                    nc.gpsimd.partition_broadcast(rb[:], rsum[:])
                    nc.gpsimd.tensor_mul(attn[2 * i][:, qc:qc + 256],
                                         ps_o[:, 0:256], rb[:, 0:256])
                    nc.gpsimd.tensor_mul(attn[2 * i + 1][:, qc:qc + 256],
                                         ps_o[:, 256:512], rb[:, 256:512])
                return emit

            def attention_steps(qi, defer=True):
                """List of step closures for chunk qi's attention. Each step
                is a small emission unit; the caller interleaves them with
                qkv/o_proj matmuls for pipelining."""
                steps = []
                qc = qi * 256
                for i in range(KVH):
                    st = {"acc": None, "ps_o": None, "po3": None}
                    q2 = qpair[i][:].rearrange("p (h s) -> p h s", h=2)
                    jstart = max(0, qc // 128 - 8)
                    jend = qc // 128 + 1
                    fulls = [j for j in range(jstart, jend)
                             if qc - j * 128 != 1024]
                    halves = ([j for j in range(jstart, jend)
                               if qc - j * 128 == 1024] + [jend])
                    tiles = ([("f", j) for j in fulls] +
                             [("h", j) for j in halves])
                    ntile = len(tiles)

                    def a_step(kind, j, idx, st=st, i=i, qc=qc, q2=q2,
                               ntile=ntile):
                        delta = qc - j * 128
                        ps_sc = scp.tile([128, 512], f32, tag="sc", name="sc")
                        if kind == "f":
                            nc.tensor.matmul(
                                ps_sc[:], kct[i][j][:],
                                q2[:, :, qc:qc + 256],
                                start=True, stop=True)
                            pb = pbp.tile([128, 512], bf16, tag="pb",
                                          name="pb")
                            if idx == 0 and delta not in (896, 0):
                                # clean first tile: exp straight into acc
                                acc = accp.tile([128, 512], bf16, tag="acc",
                                                name="acc")
                                nc.scalar.activation(acc[:], ps_sc[:], Exp,
                                                     bias=0.0, scale=SCALE)
                                st["acc"] = acc
                                st["pb"] = acc
                                return
                            nc.scalar.activation(pb[:], ps_sc[:], Exp,
                                                 bias=0.0, scale=SCALE)
                            if delta in (896, 0):
                                moff = M896 if delta == 896 else M0
                                if idx == 0:
                                    acc = accp.tile([128, 512], bf16,
                                                    tag="acc", name="acc")
                                    nc.vector.scalar_tensor_tensor(
                                        acc[:], pb[:], 1.0,
                                        msk_sb[:, moff:moff + 512],
                                        mult, mult)
                                    # pv must use masked probs
                                    st["acc"] = acc
                                    st["pb"] = acc
                                    return
                                nc.vector.scalar_tensor_tensor(
                                    pb[:], pb[:], 1.0,
                                    msk_sb[:, moff:moff + 512], mult, mult)
                            nacc = accp.tile([128, 512], bf16, tag="acc",
                                             name="acc")
                            nc.vector.scalar_tensor_tensor(
                                nacc[:], pb[:], 1.0, st["acc"][:], mult, add)
                            st["acc"] = nacc
                            st["pb"] = pb
                        else:
                            qoff = 0 if delta == 1024 else 128
                            moff = M1024H if delta == 1024 else MM128H
                            ps3 = ps_sc[:].rearrange("p (h q) -> p h q", h=2)
                            nc.tensor.matmul(
                                ps3[:, :, 0:128], kct[i][j][:],
                                q2[:, :, qc + qoff:qc + qoff + 128],
                                start=True, stop=True)
                            pb = pbp.tile([128, 512], bf16, tag="pb",
                                          name="pb")
                            pb3 = pb[:].rearrange("p (h q) -> p h q", h=2)
                            nc.scalar.activation(pb3[:, :, 0:128],
                                                 ps3[:, :, 0:128], Exp,
                                                 bias=0.0, scale=SCALE)
                            m3 = msk_sb[:, moff:moff + 256].rearrange(
                                "p (h q) -> p h q", h=2)
                            nc.vector.scalar_tensor_tensor(
                                pb3[:, :, 0:128], pb3[:, :, 0:128], 1.0,
                                m3[:, :, :], mult, mult)
                            last_fold = (idx == ntile - 1)
                            pool = accfp if last_fold else accp
                            tag = "accf" if last_fold else "acc"
                            nacc = pool.tile([128, 512], bf16, tag=tag,
                                             name=tag)
                            acc3 = st["acc"][:].rearrange(
                                "p (h q) -> p h q", h=2)
                            nacc3 = nacc[:].rearrange("p (h q) -> p h q", h=2)
                            nc.vector.scalar_tensor_tensor(
                                nacc3[:, :, qoff:qoff + 128],
                                pb3[:, :, 0:128], 1.0,
                                acc3[:, :, qoff:qoff + 128], mult, add)
                            oq = 128 - qoff
                            nc.vector.tensor_copy(
                                nacc3[:, :, oq:oq + 128],
                                acc3[:, :, oq:oq + 128])
                            st["acc"] = nacc
                            st["pb"] = pb

                    def b_step(kind, j, idx, st=st, i=i, ntile=ntile, qc=qc):
                        first = (idx == 0)
                        last = (idx == ntile - 1)
                        if first:
                            st["ps_o"] = pvp.tile([128, 512], f32, tag="pv",
                                                  name="pv")
                            st["po3"] = st["ps_o"][:].rearrange(
                                "p (h q) -> p h q", h=2)
                        pb = st[f"pb{idx}"]
                        if kind == "f":
                            nc.tensor.matmul(st["ps_o"][:], vt[i][j][:],
                                             pb[:], start=first, stop=last)
                        else:
                            delta = qc - j * 128
                            qoff = 0 if delta == 1024 else 128
                            pb3 = pb[:].rearrange("p (h q) -> p h q", h=2)
                            nc.tensor.matmul(
                                st["po3"][:, :, qoff:qoff + 128],
                                vt[i][j][:], pb3[:, :, 0:128],
                                start=False, stop=last)

                    # assemble: a0,a1,[pending sums],b0,a2,b1,a3,...,bn-2,bn-1
                    def make_a(kind, j, idx, a_step=a_step, st=st):
                        def f():
                            a_step(kind, j, idx)
                            st[f"pb{idx}"] = st["pb"]
                        return f

                    def make_b(kind, j, idx, b_step=b_step):
                        return lambda: b_step(kind, j, idx)

                    seq = []
                    seq.append(make_a(*tiles[0], 0))
                    if ntile > 1:
                        seq.append(make_a(*tiles[1], 1))
                    seq.extend(pending)
                    del pending[:]
                    seq.append(make_b(*tiles[0], 0))
                    for idx in range(2, ntile):
                        seq.append(make_a(*tiles[idx], idx))
                        seq.append(make_b(*tiles[idx - 1], idx - 1))
                    if ntile > 1:
                        seq.append(make_b(*tiles[ntile - 1], ntile - 1))

                    if defer:
                        def defer_sum(st=st, qc=qc, i=i):
                            pending.append(
                                make_sum_step(st["acc"], st["ps_o"], qc, i))
                        seq.append(defer_sum)
                    else:
                        def now_sum(st=st, qc=qc, i=i):
                            make_sum_step(st["acc"], st["ps_o"], qc, i)()
                        seq.append(now_sum)
                    steps.extend(seq)
                return steps

            def oproj_units(qi):
                """8 units; unit = 2 oc blocks x 4 accumulating matmuls over
                a [128,256] token block, sharing one psum bank, then one
                psum->sbuf copy + 2 DMAs out."""
                units = []
                for ocp in range(NK // 2):
                    def unit(ocp=ocp, qi=qi):
                        ps_y = mm.tile([128, 512], f32, tag="qkv", name="op")
                        for half in range(2):
                            oc = 2 * ocp + half
                            for d in range(QH):
                                nc.tensor.matmul(
                                    ps_y[:, half * 256:(half + 1) * 256],
                                    wo_sb[:, d * H + oc * 128:
                                          d * H + (oc + 1) * 128],
                                    attn[d][:, qi * 256:(qi + 1) * 256],
                                    start=(d == 0), stop=(d == QH - 1))
                        yb = ybp.tile([128, 512], f32, tag="yb", name="yb")
                        nc.scalar.copy(yb[:], ps_y[:])
                        nc.sync.dma_start(
                            out=yT_d[ocp * 256:(ocp + 1) * 256,
                                     qi * 256:(qi + 1) * 256].rearrange(
                                "(o p) s -> p o s", o=2),
                            in_=yb[:].rearrange("p (o s) -> p o s", o=2))
                    units.append(unit)
                return units

            krprev = [None]
            vslvprev = [None]

            def rope_and_conv(t, psQ, psK, cssn_t):
                csl = cssn_t[:, 0:512]
                snl = cssn_t[:, 512:1024]
                kr = rl.tile([128, 512], bf16, tag="kr", name="kr")
                for cp in range(3):
                    src = psQ[cp] if cp < 2 else psK
                    if cp < 2:
                        dst3 = qpair[cp][:].rearrange(
                            "p (h s) -> p h s", h=2)[:, :,
                                                     t * 256:t * 256 + 256]
                    else:
                        dst3 = kr[:].rearrange("p (h s) -> p h s", h=2)
                    e1 = eb.tile([128, 512], f32, tag="e1", name="e1")
                    e2 = eb.tile([128, 512], f32, tag="e2", name="e2")
                    nc.vector.tensor_mul(e1[:], src[:], csl)
                    nc.vector.tensor_mul(e2[:], src[:], snl)
                    e13 = e1[:].rearrange("p (h s) -> p h s", h=2)
                    e23 = e2[:].rearrange("p (h s) -> p h s", h=2)
                    nc.gpsimd.tensor_sub(dst3[0:64, :, :],
                                         e13[0:64, :, :], e23[64:128, :, :])
                    nc.gpsimd.tensor_add(dst3[64:128, :, :],
                                         e23[0:64, :, :], e13[64:128, :, :])
                # K conv: kconv = kr + (w0k/w1k) * kr_prev (w1k in weights)
                for i in range(KVH):
                    r = cw_sb[:, i:i + 1]
                    b = i * 256
                    nc.vector.scalar_tensor_tensor(
                        kct[i][2 * t][:, 1:128],
                        kr[:, b:b + 127], r,
                        kr[:, b + 1:b + 128], mult, add)
                    nc.vector.scalar_tensor_tensor(
                        kct[i][2 * t + 1][:, 0:128],
                        kr[:, b + 127:b + 255], r,
                        kr[:, b + 128:b + 256], mult, add)
                    if t == 0:
                        nc.vector.tensor_copy(kct[i][0][:, 0:1],
                                              kr[:, b:b + 1])
                    else:
                        nc.vector.scalar_tensor_tensor(
                            kct[i][2 * t][:, 0:1],
                            krprev[0][:, b + 255:b + 256], r,
                            kr[:, b:b + 1], mult, add)
                krprev[0] = kr

            def vconv(t, psV):
                vslv = rl.tile([1, 256], f32, tag="vslv", name="vslv")
                for i in range(KVH):
                    r = cw_sb[:, 2 + i:3 + i]
                    for sub in range(2):
                        dst = vt[i][2 * t + sub]
                        src = psV[:, sub * 256 + i * 128:
                                  sub * 256 + (i + 1) * 128]
                        nc.vector.scalar_tensor_tensor(
                            dst[1:128, :], src[0:127, :], r[0:127, :],
                            src[1:128, :], mult, add)
                        if sub == 1:
                            nc.vector.scalar_tensor_tensor(
                                dst[0:1, :],
                                psV[127:128, i * 128:(i + 1) * 128],
                                r[0:1, :], src[0:1, :], mult, add)
                        elif t > 0:
                            nc.vector.scalar_tensor_tensor(
                                dst[0:1, :],
                                vslvprev[0][0:1, i * 128:(i + 1) * 128],
                                r[0:1, :], src[0:1, :], mult, add)
                        else:
                            nc.vector.tensor_copy(dst[0:1, :], src[0:1, :])
                nc.vector.tensor_copy(vslv[:], psV[127:128, 256:512])
                vslvprev[0] = vslv

            # ---------------- main software-pipelined loop ----------------
            for t in range(NT + 1):
                steps = (attention_steps(t - 1, defer=(t - 1 != NT - 1))
                         if t >= 1 else [])
                units = oproj_units(t - 2) if t >= 2 else []
                if t < NT:
                    fill = steps + units
                else:
                    # tail: first steps carry pending softmax sums that
                    # o_proj units consume; then alternate for pipelining
                    fill = steps[:4]
                    rest = steps[4:]
                    for u in range(max(len(units), len(rest))):
                        if u < len(units):
                            fill.append(units[u])
                        if u < len(rest):
                            fill.append(rest[u])
                si = 0

                if t < NT:
                    if t + 1 < NT:
                        nhtile = bht.tile([128, NK * 256], bf16, tag="htc",
                                          name="htc")
                        nc.sync.dma_start(
                            out=nhtile[:].rearrange("p (k s) -> p k s", k=NK),
                            in_=hT_d[:, (t + 1) * 256:(t + 2) * 256].rearrange(
                                "(k p) s -> p k s", k=NK))
                        ncssn = load_cssn(t + 1)
                    if t == 0:
                        nc.sync.dma_start(
                            out=wo_sb[:].rearrange("p (d c) -> p d c", d=QH),
                            in_=wo_d[:, :].rearrange("(d p) c -> p d c",
                                                     d=QH))
                    # pass A: Q pairs + K pair, k-outer with 3 open psums
                    psQ = [mm.tile([128, 512], f32, tag="qkv",
                                   name=f"psq{cp}") for cp in range(2)]
                    psK = mm.tile([128, 512], f32, tag="qkv", name="psk")
                    for k in range(NK):
                        hk = htile[:, k * 256:(k + 1) * 256]
                        for cp in range(2):
                            for h in range(2):
                                c = 2 * cp + h
                                nc.tensor.matmul(
                                    psQ[cp][:, h * 256:(h + 1) * 256],
                                    wfs(k, c * 128, (c + 1) * 128), hk,
                                    start=(k == 0), stop=(k == NK - 1))
                        for h in range(2):
                            nc.tensor.matmul(
                                psK[:, h * 256:(h + 1) * 256],
                                wfs(k, (4 + h) * 128, (5 + h) * 128), hk,
                                start=(k == 0), stop=(k == NK - 1))
                        # interleave attention/o_proj emission units
                        dly = 3 if t == 1 else 0
                        want = max(0, k + 1 - dly) * len(fill) // (NK - dly)
                        while si < min(want, len(fill)):
                            fill[si]()
                            si += 1
                    # rope Q/K + K conv first: their emitted reads release
                    # the psQ bufs so psV can take one over
                    rope_and_conv(t, psQ, psK, cssn)
                    # pass B: V transposed (hT stationary) into a freed buf
                    psV = mm.tile([128, 512], f32, tag="qkv", name="psv")
                    for k in range(NK):
                        for sub in range(2):
                            nc.tensor.matmul(
                                psV[:, sub * 256:(sub + 1) * 256],
                                htile[:, k * 256 + sub * 128:
                                      k * 256 + (sub + 1) * 128],
                                wfs(k, 768, 1024),
                                start=(k == 0), stop=(k == NK - 1))
                    vconv(t, psV)
                    if t + 1 < NT:
                        htile = nhtile
                        cssn = ncssn
                while si < len(fill):
                    fill[si]()
                    si += 1
            # last pending softmax denominator (chunk 7, i=1), then its o_proj
            for p in pending:
                p()
            del pending[:]
            for u in oproj_units(NT - 1):
                u()

    nc.finalize()
    return nc


def _host_inputs(hidden, W_pack, W_o, conv_k, conv_v):
    """Per-core input maps."""
    bf16 = ml_dtypes.bfloat16
    pos = np.arange(S, dtype=np.float64)
    inv_freq = 1.0 / (THETA ** (np.arange(0, HD, 2, dtype=np.float64) / HD))
    freqs = np.outer(pos, inv_freq)                       # (S, 64)
    cos = np.cos(freqs).T.astype(np.float32)              # (64, S)
    sin = np.sin(freqs).T.astype(np.float32)
    cos = np.concatenate([cos, cos], axis=0)              # (128, S)
    sin = np.concatenate([sin, sin], axis=0)
    # per chunk: [cos dup x2 heads (512) | sin dup x2 heads (512)]
    cs = np.broadcast_to(
        cos.reshape(128, NT, 1, 256), (128, NT, 2, 256)).reshape(128, NT, 512)
    sn = np.broadcast_to(
        sin.reshape(128, NT, 1, 256), (128, NT, 2, 256)).reshape(128, NT, 512)
    csn = np.concatenate([cs, sn], axis=2).reshape(128, -1)
    csn = np.ascontiguousarray(csn).astype(np.float32)

    kk = np.arange(128)[:, None]
    qq = np.arange(128)[None, :]
    qq2 = np.arange(256)[None, :]

    def double(m):
        return np.concatenate([m, m], axis=1)
    m1024h = double(qq < kk)                       # [128, 256]
    m896 = double(qq2 - kk < 128)                  # [128, 512]
    m0 = double(qq2 >= kk)                         # [128, 512]
    mm128h = double(qq >= kk)                      # [128, 256]
    msk = np.concatenate([m1024h, m896, m0, mm128h],
                         axis=1).astype(bf16)      # [128, 1536]

    one = np.ones((128, 1), bf16)

    in_maps = []
    for c in range(NCORES):
        b, g = c // TP, c % TP
        hT = np.ascontiguousarray(hidden[b].T).astype(bf16)
        wq = W_pack[:, g * 512:(g + 1) * 512]
        wk = W_pack[:, NH * HD + 2 * g * 128: NH * HD + (2 * g + 2) * 128]
        wv = W_pack[:, NH * HD + NKV * HD + 2 * g * 128:
                    NH * HD + NKV * HD + (2 * g + 2) * 128]
        # fold conv w1 into Wk/Wv (rope is linear; conv comes after rope)
        wk = wk.copy()
        wv = wv.copy()
        for i in range(KVH):
            wk[:, i * 128:(i + 1) * 128] *= conv_k[2 * g + i, 1]
            wv[:, i * 128:(i + 1) * 128] *= conv_v[2 * g + i, 1]
        wpk = np.ascontiguousarray(
            np.concatenate([wq, wk, wv], axis=1)).astype(bf16)
        wo = np.ascontiguousarray(
            W_o[g * 512:(g + 1) * 512, :]).astype(bf16)
        cwv = np.empty(4, np.float32)
        for i in range(KVH):
            cwv[i] = conv_k[2 * g + i, 0] / conv_k[2 * g + i, 1]
            cwv[2 + i] = conv_v[2 * g + i, 0] / conv_v[2 * g + i, 1]
        cw = np.broadcast_to(cwv, (128, 4)).copy()
        in_maps.append({
            "hT": hT, "wpk": wpk, "wo": wo, "csn": csn,
            "cw": cw, "msk": msk, "one": one,
        })
    return in_maps


def run_cores(in_maps, trace=False, **kw):
    from concourse.bass_utils import run_bass_kernel_spmd
    if "nc" not in _CACHE:
        _CACHE["nc"] = _build_program()
    return run_bass_kernel_spmd(_CACHE["nc"], in_maps, list(range(NCORES)),
                                trace=trace, **kw)


def kernel(hidden, W_pack, W_o, conv_k, conv_v):
    hidden = np.asarray(hidden, np.float32)
    W_pack = np.asarray(W_pack, np.float32)
    W_o = np.asarray(W_o, np.float32)
    conv_k = np.asarray(conv_k, np.float32)
    conv_v = np.asarray(conv_v, np.float32)
    in_maps = _host_inputs(hidden, W_pack, W_o, conv_k, conv_v)
    res = run_cores(in_maps)
    out = np.zeros((B, S, H), np.float32)
    for c in range(NCORES):
        b = c // TP
        out[b] += res.results[c]["yT"].T
    return out
